# revision 16
# baseline (speedup 1.0000x reference)
"""Causal attention (B=4, L=4096, D=2048, HD=128) on 8 TRN2 NeuronCores.

Sharding: 8 cores = 4 batches x 2 fold-halves. Core c handles batch b=c//2
and query blocks {i, 3-i} (1024 rows each) where i=c%2 — the "fold" split
balances causal attention work exactly across the two cores of a batch.
Each core recomputes K/V for all 4096 keys of its batch (no collectives).

The on-device program is identical on all cores (SPMD); per-core behavior
comes only from the data: a block-permuted repacked input xr and two
slot-bias vectors that enable/disable the two fold-dependent key blocks
(bias 0 keeps scores, bias -50 drives exp() to ~1e-22, i.e. masks).

v2 changes vs v1:
  - host repacks x and the weights so every DMA has >=4KB contiguous
    per-partition lines (one DMA per 512-col xt tile instead of 4, one
    per weight); output is written bf16 in [128, qb, dt, 512] layout,
    two DMAs per 512-query block instead of 16.
  - score tiles are computed in PAIRS ([128,1024] PSUM spanning 2 banks)
    so one ACT exp call covers 2 k-tiles, amortizing the 352-cycle ACT
    fixed overhead (1147ns/pair vs 2x720ns).
  - the est tree-adds for row-sums moved from gpsimd to DVE on pairs.
  - attention units take a "filler" generator (projection or out-proj
    chunks) and interleave one chunk per score pair so PE never waits
    on the ACT exp pipeline.

Layouts (partition dim first):
  xr      [128, 8, 16, 512]  x[b].T block-permuted: [p, rb, dt, col]
  Qt, Kt  [HD=128, Lq/Lk]    projections, head dim on partitions
  v_s     [k, HD] slabs      natural V per 128-key tile (PE transpose)
  scores  [k=128, 1024]      two k-tiles per PSUM pair tile; exp on ACT
  outT    [128, 4, 16, 512]  bf16 [p, qb, dt, col]; host divides by
                             row-sums, transposes back, adds bo
"""

import numpy as np
import ml_dtypes

B, L, D, HD = 4, 4096, 2048, 128
BLK = 1024            # fold block (4 per batch)
LQ = 2 * BLK          # queries per core
LK = L                # keys per core
ND = D // 128         # 16 d-tiles
NRB = LK // 512       # 8 column blocks for projections
NEG = -50.0           # slot-disable bias (exp(x-50) ~ 0)
MASKVAL = -30000.0    # intra-tile causal mask additive value

_cached = {}


def _build_program():
    import concourse.bass as bass
    import concourse.tile as tile
    from concourse import bacc, mybir
    from concourse.masks import make_identity

    f32 = mybir.dt.float32
    bf16 = mybir.dt.bfloat16
    nc = bacc.Bacc("TRN2", target_bir_lowering=False, debug=False)

    xr_d = nc.dram_tensor("xr", (128, NRB, ND, 512), bf16,
                          kind="ExternalInput")
    wq_d = nc.dram_tensor("wq", (128, ND, 128), bf16, kind="ExternalInput")
    wk_d = nc.dram_tensor("wk", (128, ND, 128), bf16, kind="ExternalInput")
    wv_d = nc.dram_tensor("wv", (128, ND, 128), bf16, kind="ExternalInput")
    wo_d = nc.dram_tensor("wo", (HD, D), bf16, kind="ExternalInput")
    bias_d = nc.dram_tensor("biases", (128, 8), f32, kind="ExternalInput")
    out_d = nc.dram_tensor("outT", (128, 4, ND, 512), bf16,
                           kind="ExternalOutput")
    rs_d = nc.dram_tensor("rowsums", (1, LQ), f32, kind="ExternalOutput")

    # phase -> list of (local_kblk, kind); kind in {"diag", "full", "bA", "bB"}
    SLOTS = {
        0: [(0, "diag"), (2, "bA")],
        1: [(0, "full"), (1, "diag"), (2, "full"), (3, "bB")],
    }

    with tile.TileContext(nc) as tc:
        with (
            tc.tile_pool(name="const", bufs=1) as cpool,
            tc.tile_pool(name="xt", bufs=3) as xtpool,
            tc.tile_pool(name="vt", bufs=3) as vtpool,
            tc.tile_pool(name="expst", bufs=6) as epool,
            tc.tile_pool(name="outsb", bufs=2) as outpool,
            tc.tile_pool(name="psum", bufs=1, space="PSUM") as psum,
        ):
            # ---- persistent SBUF tensors ----
            wq_s = cpool.tile([128, ND, 128], bf16, tag="wq")
            wk_s = cpool.tile([128, ND, 128], bf16, tag="wk")
            wv_s = cpool.tile([128, ND, 128], bf16, tag="wv")
            wo_s = cpool.tile([128, D], bf16, tag="wo")
            bias_s = cpool.tile([128, 8], f32, tag="biases")
            kt_s = cpool.tile([128, LK], bf16, tag="kt")
            qt_s = cpool.tile([128, LQ], bf16, tag="qt")
            v_s = cpool.tile([128, LK], bf16, tag="v")
            ones_s = cpool.tile([128, 1], bf16, tag="ones")
            rs_s = cpool.tile([1, LQ], f32, tag="rs")
            masks_s = cpool.tile([128, 4 * 512], f32, tag="masks")
            ot_s = cpool.tile([128, LQ], bf16, tag="ot")
            identb_s = cpool.tile([128, 128], bf16, tag="identb")

            # first xt block + wk first so PE can start ASAP; xt1 ahead of
            # wv/wq so rb1's K can follow rb0 without a DMA underrun; wo is
            # deferred (not needed until the first out-projection).
            xts = {}
            nc.sync.dma_start(wk_s[:, 0:4], wk_d.ap()[:, 0:4])
            xts[0] = xtpool.tile([128, ND, 512], bf16, tag="xt", name="xt")
            nc.sync.dma_start(xts[0][:, 0:2, :], xr_d.ap()[:, 0, 0:2, :])
            nc.sync.dma_start(wk_s[:, 4:ND], wk_d.ap()[:, 4:ND])
            nc.sync.dma_start(xts[0][:, 2:4, :], xr_d.ap()[:, 0, 2:4, :])
            nc.sync.dma_start(xts[0][:, 4:8, :], xr_d.ap()[:, 0, 4:8, :])
            nc.sync.dma_start(xts[0][:, 8:12, :], xr_d.ap()[:, 0, 8:12, :])
            nc.sync.dma_start(xts[0][:, 12:16, :], xr_d.ap()[:, 0, 12:16, :])
            nc.sync.dma_start(wv_s[:], wv_d.ap())
            nc.sync.dma_start(wq_s[:], wq_d.ap())
            nc.sync.dma_start(bias_s[:], bias_d.ap())
            xts[1] = xtpool.tile([128, ND, 512], bf16, tag="xt", name="xt")
            nc.sync.dma_start(xts[1][:], xr_d.ap()[:, 1])

            make_identity(nc, identb_s[:])
            nc.gpsimd.memset(ones_s[:], 1.0)
            # 4 causal mask tiles for relative offsets delta = 0,128,256,384:
            # keep 0 where q_free >= k_part + delta, else MASKVAL
            nc.gpsimd.memset(masks_s[:], 0.0)
            for m in range(4):
                nc.gpsimd.affine_select(
                    out=masks_s[:, m * 512:(m + 1) * 512],
                    in_=masks_s[:, m * 512:(m + 1) * 512],
                    compare_op=mybir.AluOpType.is_ge,
                    fill=MASKVAL,
                    base=-(m * 128),
                    channel_multiplier=-1,
                    pattern=[[1, 512]],
                )

            bq_ap = bias_s[:, 0:1]
            bk_ap = bias_s[:, 1:2]
            bv_ap = bias_s[:, 2:3]
            slot_bias = {"bA": bias_s[:, 3:4], "bB": bias_s[:, 4:5]}

            def prefetch(rb):
                xts[rb] = xtpool.tile([128, ND, 512], bf16, tag="xt",
                                      name="xt")
                nc.sync.dma_start(xts[rb][:], xr_d.ap()[:, rb])

            def emit_rb_gen(rb, prefetch_rb=None):
                """Projections for one 512-wide column block of xr.
                Yields between ~1us chunks so it can fill attention gaps."""
                xt = xts.pop(rb)
                if prefetch_rb is not None:
                    prefetch(prefetch_rb)
                cs = slice(rb * 512, (rb + 1) * 512)

                pk = psum.tile([128, 512], f32, tag="acc512", bufs=2,
                               name="pk")
                for dt in range(ND):
                    nc.tensor.matmul(
                        pk[:], wk_s[:, dt, :], xt[:, dt, :],
                        start=(dt == 0), stop=(dt == ND - 1),
                    )
                    if dt % 4 == 3:
                        yield
                nc.vector.tensor_scalar_add(kt_s[:, cs], pk[:], bk_ap)

                pv = psum.tile([128, 512], f32, tag="acc512", bufs=2,
                               name="pv")
                for dt in range(ND):
                    nc.tensor.matmul(
                        pv[:], wv_s[:, dt, :], xt[:, dt, :],
                        start=(dt == 0), stop=(dt == ND - 1),
                    )
                    if dt % 4 == 3:
                        yield
                vt_tmp = vtpool.tile([128, 512], bf16, tag="vt_tmp")
                nc.vector.tensor_scalar_add(vt_tmp[:], pv[:], bv_ap)
                for s in range(4):
                    ktile = rb * 4 + s
                    vp = psum.tile([128, 128], bf16, tag="acc512", bufs=2,
                                   name="vp")
                    nc.tensor.transpose(
                        vp[:], vt_tmp[:, s * 128:(s + 1) * 128], identb_s[:]
                    )
                    nc.vector.tensor_copy(
                        v_s[:, ktile * 128:(ktile + 1) * 128], vp[:]
                    )
                yield

                if rb < LQ // 512:
                    pq = psum.tile([128, 512], f32, tag="acc512", bufs=2,
                                   name="pq")
                    for dt in range(ND):
                        nc.tensor.matmul(
                            pq[:], wq_s[:, dt, :], xt[:, dt, :],
                            start=(dt == 0), stop=(dt == ND - 1),
                        )
                        if dt % 4 == 3:
                            yield
                    nc.vector.tensor_scalar_add(qt_s[:, cs], pq[:], bq_ap)

            def emit_rb(rb, prefetch_rb=None):
                for _ in emit_rb_gen(rb, prefetch_rb):
                    pass

            def build_pairs(phase, u):
                """Pairs of k-tiles sharing one exp: (kt_a, kt_b, mask_off,
                bkey). mask_off indexes masks_s[:, off:off+1024]."""
                pairs = []
                for kblk, kind in SLOTS[phase]:
                    tiles = []
                    for t in range(8):
                        if kind == "diag":
                            drel = t * 128 - u * 512
                            if drel >= 512:
                                continue
                            midx = drel // 128 if drel >= 0 else None
                            tiles.append((kblk * 8 + t, midx))
                        else:
                            tiles.append((kblk * 8 + t, None))
                    bkey = kind if kind in slot_bias else None
                    # tiles with masks come in runs of consecutive midx
                    i = 0
                    while i < len(tiles):
                        (ta, ma), (tb, mb) = tiles[i], tiles[i + 1]
                        assert (ma is None) == (mb is None)
                        moff = None if ma is None else ma * 512
                        pairs.append((ta, tb, moff, bkey))
                        i += 2
                return pairs

            def emit_attn_u(phase, u, filler=None, nfill=1):
                q0 = phase * BLK + u * 512
                pairs = build_pairs(phase, u)
                n = len(pairs)
                ngroups = n // 2
                ot_acc = psum.tile([128, 512], f32, tag="otacc", bufs=1,
                                   name="ot_acc")
                rs_acc = psum.tile([1, 512], f32, tag="rs", bufs=1,
                                   name="rs_acc")
                ests = [None] * n

                def emit_pair(pi):
                    ta, tb, moff, bkey = pairs[pi]
                    stp = psum.tile([128, 1024], f32, tag="stp", bufs=2,
                                    name="stp")
                    nc.tensor.matmul(
                        stp[:, 0:512],
                        kt_s[:, ta * 128:(ta + 1) * 128],
                        qt_s[:, q0:q0 + 512],
                        start=True, stop=True,
                    )
                    nc.tensor.matmul(
                        stp[:, 512:1024],
                        kt_s[:, tb * 128:(tb + 1) * 128],
                        qt_s[:, q0:q0 + 512],
                        start=True, stop=True,
                    )
                    if moff is not None:
                        nc.vector.tensor_add(
                            stp[:], stp[:], masks_s[:, moff:moff + 1024]
                        )
                    est = epool.tile([128, 1024], bf16, tag="est")
                    nc.scalar.activation(
                        est[:], stp[:],
                        mybir.ActivationFunctionType.Exp,
                        bias=slot_bias[bkey] if bkey else 0.0,
                    )
                    ests[pi] = est

                if filler is not None:
                    next(filler, None)
                emit_pair(0)
                if filler is not None:
                    next(filler, None)
                if n > 1:
                    emit_pair(1)
                for pi in range(n):
                    ta, tb, moff, bkey = pairs[pi]
                    if pi + 2 < n:
                        emit_pair(pi + 2)
                    if filler is not None:
                        for _ in range(nfill):
                            next(filler, None)
                    est = ests[pi]
                    nc.tensor.matmul(
                        ot_acc[:],
                        v_s[:, ta * 128:(ta + 1) * 128],
                        est[:, 0:512],
                        start=(pi == 0), stop=False,
                    )
                    nc.tensor.matmul(
                        ot_acc[:],
                        v_s[:, tb * 128:(tb + 1) * 128],
                        est[:, 512:1024],
                        start=False, stop=(pi == n - 1),
                    )
                    if pi % 2 == 1:
                        g = pi // 2
                        esum = epool.tile([128, 1024], bf16, tag="esum",
                                          name="esum")
                        nc.vector.tensor_add(
                            esum[:], ests[pi - 1][:], est[:]
                        )
                        fold = epool.tile([128, 512], bf16, tag="fold",
                                          name="fold")
                        nc.vector.tensor_add(
                            fold[:], esum[:, 0:512], esum[:, 512:1024]
                        )
                        nc.tensor.matmul(
                            rs_acc[:], ones_s[:], fold[:],
                            start=(g == 0), stop=(g == ngroups - 1),
                        )

                qb = phase * 2 + u
                nc.vector.tensor_copy(
                    ot_s[:, qb * 512:(qb + 1) * 512], ot_acc[:]
                )
                nc.vector.tensor_copy(
                    rs_s[:, qb * 512:(qb + 1) * 512], rs_acc[:]
                )
                if filler is not None:
                    for _ in filler:  # drain unconsumed filler chunks
                        pass

            def outproj_gen(qb, on_act=False, nstores=2):
                """Out-projection for one 512-query block into a bf16 slab.
                Stores go on the gpsimd SWDGE queue so they never wait
                behind input-prefetch WAR stalls on the sync queue. Yields
                per dt chunk. on_act alternates copies onto ACT (only for
                regions where ACT is not running exp)."""
                slab = outpool.tile([128, ND, 512], bf16, tag="oslab",
                                    name="oslab")
                per = ND // nstores
                po = None
                for dt in range(ND):
                    if dt % 2 == 0:
                        # [128,1024] f32 tile on the stp tag used as two po
                        # slots -> 4 drains in flight (stp bufs=2)
                        po = psum.tile([128, 1024], f32, tag="stp",
                                       bufs=2, name="po")
                    sl = slice((dt % 2) * 512, (dt % 2) * 512 + 512)
                    nc.tensor.matmul(
                        po[:, sl],
                        wo_s[:, dt * 128:(dt + 1) * 128],
                        ot_s[:, qb * 512:(qb + 1) * 512],
                        start=True, stop=True,
                    )
                    if on_act and dt % 2 == 1:
                        nc.scalar.activation(
                            slab[:, dt, :], po[:, sl],
                            mybir.ActivationFunctionType.Copy,
                        )
                    else:
                        nc.vector.tensor_copy(slab[:, dt, :], po[:, sl])
                    if dt % per == per - 1:
                        s = dt + 1 - per
                        nc.gpsimd.dma_start(
                            out_d.ap()[:, qb, s:dt + 1], slab[:, s:dt + 1]
                        )
                    yield

            def emit_outproj(qb, on_act=False, nstores=2):
                for _ in outproj_gen(qb, on_act, nstores):
                    pass

            def chain(*gens):
                for g in gens:
                    for x in g:
                        yield x

            def interleave(*gens):
                live = list(gens)
                while live:
                    for g in list(live):
                        try:
                            next(g)
                        except StopIteration:
                            live.remove(g)

            # ---- interleaved schedule ----
            # phase 0 needs local kblks 0 (rbs 0,1) and 2 (rbs 4,5) plus
            # Qt[0:1024) (rbs 0,1); phase 1 needs everything.
            emit_rb(0, prefetch_rb=4)
            nc.sync.dma_start(wo_s[:], wo_d.ap())
            emit_rb(1, prefetch_rb=5)
            emit_rb(4, prefetch_rb=2)
            emit_rb(5, prefetch_rb=3)
            emit_attn_u(0, 0, filler=emit_rb_gen(2, prefetch_rb=6))
            emit_attn_u(0, 1, filler=emit_rb_gen(3, prefetch_rb=7))
            # out-proj of qb0 rides along rb6/rb7: PE adds its matmuls while
            # its PSUM drains use the exp-free DVE/ACT window
            interleave(chain(emit_rb_gen(6), emit_rb_gen(7)),
                       outproj_gen(0, on_act=True))
            emit_attn_u(1, 0, filler=outproj_gen(1))
            emit_attn_u(1, 1, filler=outproj_gen(2))
            emit_outproj(3, on_act=True, nstores=4)
            nc.sync.dma_start(rs_d.ap(), rs_s[:])

    nc.compile()
    return nc


def _get_program():
    if "nc" not in _cached:
        _cached["nc"] = _build_program()
    return _cached["nc"]


def _perm_blocks(i):
    # local order [qA, qB, o1, o2]
    return [0, 3, 1, 2] if i == 0 else [1, 2, 0, 3]


def _repack_w(w):
    # (D, HD) -> [128, ND, 128] with per-partition contiguous lines
    return np.ascontiguousarray(
        w.reshape(ND, 128, HD).transpose(1, 0, 2)
    ).astype(ml_dtypes.bfloat16)


def make_in_maps(x, Wq, bq, Wk, bk, Wv, bv, Wo, bo):
    scale = 1.0 / np.sqrt(np.float32(HD))
    wq_r = _repack_w((Wq * scale).astype(np.float32))
    wk_r = _repack_w(Wk.astype(np.float32))
    wv_r = _repack_w(Wv.astype(np.float32))
    bq_s = (bq * scale).astype(np.float32)
    in_maps = []
    for c in range(8):
        i, b = c % 2, c // 2
        perm = _perm_blocks(i)
        xbT = x[b].T  # (D, L) view
        xT = np.concatenate(
            [xbT[:, p * BLK:(p + 1) * BLK] for p in perm], axis=1
        )
        # (D, L) -> [128, NRB, ND, 512]: xr[p, rb, dt, c] = xT[dt*128+p,
        # rb*512+c]
        xr = np.ascontiguousarray(
            xT.reshape(ND, 128, NRB, 512).transpose(1, 2, 0, 3)
        ).astype(ml_dtypes.bfloat16)
        biases = np.zeros((128, 8), np.float32)
        biases[:, 0] = bq_s
        biases[:, 1] = bk.astype(np.float32)
        biases[:, 2] = bv.astype(np.float32)
        biases[:, 3] = NEG if i == 0 else 0.0   # phase A, slot kblk=2
        biases[:, 4] = 0.0 if i == 0 else NEG   # phase B, slot kblk=3
        in_maps.append({
            "xr": xr,
            "wq": wq_r,
            "wk": wk_r,
            "wv": wv_r,
            "wo": Wo.astype(ml_dtypes.bfloat16),
            "biases": biases,
        })
    return in_maps


def assemble_output(results, bo):
    out = np.empty((B, L, D), np.float32)
    for c in range(8):
        i, b = c % 2, c // 2
        perm = _perm_blocks(i)
        arr = np.asarray(results[c]["outT"], dtype=np.float32)
        # [128, 4, ND, 512] -> (D, LQ)
        outT = arr.transpose(2, 0, 1, 3).reshape(D, LQ)
        outT /= np.asarray(results[c]["rowsums"], dtype=np.float32)
        qA, qB = perm[0], perm[1]
        out[b, qA * BLK:(qA + 1) * BLK, :] = outT[:, 0:BLK].T
        out[b, qB * BLK:(qB + 1) * BLK, :] = outT[:, BLK:2 * BLK].T
    out += bo.astype(np.float32)
    return out


def kernel(x, Wq, bq, Wk, bk, Wv, bv, Wo, bo):
    from concourse.bass_utils import run_bass_kernel_spmd

    nc = _get_program()
    in_maps = make_in_maps(
        np.asarray(x), np.asarray(Wq), np.asarray(bq), np.asarray(Wk),
        np.asarray(bk), np.asarray(Wv), np.asarray(bv), np.asarray(Wo),
        np.asarray(bo),
    )
    res = run_bass_kernel_spmd(nc, in_maps, core_ids=list(range(8)))
    return assemble_output(res.results, np.asarray(bo))


# revision 18
# speedup vs baseline: 1.0297x; 1.0297x over previous
"""Causal attention (B=4, L=4096, D=2048, HD=128) on 8 TRN2 NeuronCores.

Sharding: 8 cores = 4 batches x 2 fold-halves. Core c handles batch b=c//2
and query blocks {i, 3-i} (1024 rows each) where i=c%2 — the "fold" split
balances causal attention work exactly across the two cores of a batch.
Each core recomputes K/V for all 4096 keys of its batch (no collectives).

The on-device program is identical on all cores (SPMD); per-core behavior
comes only from the data: a block-permuted repacked input xr and two
slot-bias vectors that enable/disable the two fold-dependent key blocks
(bias 0 keeps scores, bias -50 drives exp() to ~1e-22, i.e. masks).

v2 changes vs v1:
  - host repacks x and the weights so every DMA has >=4KB contiguous
    per-partition lines (one DMA per 512-col xt tile instead of 4, one
    per weight); output is written bf16 in [128, qb, dt, 512] layout,
    two DMAs per 512-query block instead of 16.
  - score tiles are computed in PAIRS ([128,1024] PSUM spanning 2 banks)
    so one ACT exp call covers 2 k-tiles, amortizing the 352-cycle ACT
    fixed overhead (1147ns/pair vs 2x720ns).
  - the est tree-adds for row-sums moved from gpsimd to DVE on pairs.
  - attention units take a "filler" generator (projection or out-proj
    chunks) and interleave one chunk per score pair so PE never waits
    on the ACT exp pipeline.

Layouts (partition dim first):
  xr      [128, 8, 16, 512]  x[b].T block-permuted: [p, rb, dt, col]
  Qt, Kt  [HD=128, Lq/Lk]    projections, head dim on partitions
  v_s     [k, HD] slabs      natural V per 128-key tile (PE transpose)
  scores  [k=128, 1024]      two k-tiles per PSUM pair tile; exp on ACT
  outT    [128, 4, 16, 512]  bf16 [p, qb, dt, col]; host divides by
                             row-sums, transposes back, adds bo
"""

import numpy as np
import ml_dtypes

B, L, D, HD = 4, 4096, 2048, 128
BLK = 1024            # fold block (4 per batch)
LQ = 2 * BLK          # queries per core
LK = L                # keys per core
ND = D // 128         # 16 d-tiles
NRB = LK // 512       # 8 column blocks for projections
NEG = -50.0           # slot-disable bias (exp(x-50) ~ 0)
MASKVAL = -30000.0    # intra-tile causal mask additive value

_cached = {}


def _build_program():
    import concourse.bass as bass
    import concourse.tile as tile
    from concourse import bacc, mybir
    from concourse.masks import make_identity

    f32 = mybir.dt.float32
    bf16 = mybir.dt.bfloat16
    nc = bacc.Bacc("TRN2", target_bir_lowering=False, debug=False)

    xr_d = nc.dram_tensor("xr", (128, NRB, ND, 512), bf16,
                          kind="ExternalInput")
    wq_d = nc.dram_tensor("wq", (128, ND, 128), bf16, kind="ExternalInput")
    wk_d = nc.dram_tensor("wk", (128, ND, 128), bf16, kind="ExternalInput")
    wv_d = nc.dram_tensor("wv", (128, ND, 128), bf16, kind="ExternalInput")
    wo_d = nc.dram_tensor("wo", (HD, D), bf16, kind="ExternalInput")
    bias_d = nc.dram_tensor("biases", (128, 8), f32, kind="ExternalInput")
    out_d = nc.dram_tensor("outT", (128, 4, ND, 512), bf16,
                           kind="ExternalOutput")
    rs_d = nc.dram_tensor("rowsums", (1, LQ), f32, kind="ExternalOutput")

    # phase -> list of (local_kblk, kind); kind in {"diag", "full", "bA", "bB"}
    SLOTS = {
        0: [(0, "diag"), (2, "bA")],
        1: [(0, "full"), (1, "diag"), (2, "full"), (3, "bB")],
    }

    with tile.TileContext(nc) as tc:
        with (
            tc.tile_pool(name="const", bufs=1) as cpool,
            tc.tile_pool(name="xt", bufs=3) as xtpool,
            tc.tile_pool(name="vt", bufs=3) as vtpool,
            tc.tile_pool(name="expst", bufs=6) as epool,
            tc.tile_pool(name="outsb", bufs=2) as outpool,
            tc.tile_pool(name="psum", bufs=1, space="PSUM") as psum,
        ):
            # ---- persistent SBUF tensors ----
            wq_s = cpool.tile([128, ND, 128], bf16, tag="wq")
            wk_s = cpool.tile([128, ND, 128], bf16, tag="wk")
            wv_s = cpool.tile([128, ND, 128], bf16, tag="wv")
            wo_s = cpool.tile([128, D], bf16, tag="wo")
            bias_s = cpool.tile([128, 8], f32, tag="biases")
            kt_s = cpool.tile([128, LK], bf16, tag="kt")
            qt_s = cpool.tile([128, LQ], bf16, tag="qt")
            v_s = cpool.tile([128, LK], bf16, tag="v")
            ones_s = cpool.tile([128, 1], bf16, tag="ones")
            rs_s = cpool.tile([1, LQ], f32, tag="rs")
            masks_s = cpool.tile([128, 4 * 512], f32, tag="masks")
            ot_s = cpool.tile([128, LQ], bf16, tag="ot")
            identb_s = cpool.tile([128, 128], bf16, tag="identb")

            # first xt block + wk first so PE can start ASAP; xt1 ahead of
            # wv/wq so rb1's K can follow rb0 without a DMA underrun; wo is
            # deferred (not needed until the first out-projection).
            xts = {}
            nc.sync.dma_start(wk_s[:, 0:4], wk_d.ap()[:, 0:4])
            xts[0] = xtpool.tile([128, ND, 512], bf16, tag="xt", name="xt")
            nc.sync.dma_start(xts[0][:, 0:2, :], xr_d.ap()[:, 0, 0:2, :])
            nc.sync.dma_start(wk_s[:, 4:ND], wk_d.ap()[:, 4:ND])
            nc.sync.dma_start(xts[0][:, 2:4, :], xr_d.ap()[:, 0, 2:4, :])
            nc.sync.dma_start(xts[0][:, 4:8, :], xr_d.ap()[:, 0, 4:8, :])
            nc.sync.dma_start(xts[0][:, 8:12, :], xr_d.ap()[:, 0, 8:12, :])
            nc.sync.dma_start(xts[0][:, 12:16, :], xr_d.ap()[:, 0, 12:16, :])
            nc.sync.dma_start(wv_s[:], wv_d.ap())
            nc.sync.dma_start(wq_s[:], wq_d.ap())
            nc.sync.dma_start(bias_s[:], bias_d.ap())
            xts[1] = xtpool.tile([128, ND, 512], bf16, tag="xt", name="xt")
            nc.sync.dma_start(xts[1][:], xr_d.ap()[:, 1])

            make_identity(nc, identb_s[:])
            nc.gpsimd.memset(ones_s[:], 1.0)
            # 4 causal mask tiles for relative offsets delta = 0,128,256,384:
            # keep 0 where q_free >= k_part + delta, else MASKVAL
            nc.gpsimd.memset(masks_s[:], 0.0)
            for m in range(4):
                nc.gpsimd.affine_select(
                    out=masks_s[:, m * 512:(m + 1) * 512],
                    in_=masks_s[:, m * 512:(m + 1) * 512],
                    compare_op=mybir.AluOpType.is_ge,
                    fill=MASKVAL,
                    base=-(m * 128),
                    channel_multiplier=-1,
                    pattern=[[1, 512]],
                )

            bq_ap = bias_s[:, 0:1]
            bk_ap = bias_s[:, 1:2]
            bv_ap = bias_s[:, 2:3]
            slot_bias = {"bA": bias_s[:, 3:4], "bB": bias_s[:, 4:5]}

            def prefetch(rb):
                xts[rb] = xtpool.tile([128, ND, 512], bf16, tag="xt",
                                      name="xt")
                nc.sync.dma_start(xts[rb][:], xr_d.ap()[:, rb])

            def emit_rb_gen(rb, prefetch_rb=None):
                """Projections for one 512-wide column block of xr.
                Yields between ~1us chunks so it can fill attention gaps."""
                xt = xts.pop(rb)
                if prefetch_rb is not None:
                    prefetch(prefetch_rb)
                cs = slice(rb * 512, (rb + 1) * 512)

                pk = psum.tile([128, 512], f32, tag="acc512", bufs=2,
                               name="pk")
                for dt in range(ND):
                    nc.tensor.matmul(
                        pk[:], wk_s[:, dt, :], xt[:, dt, :],
                        start=(dt == 0), stop=(dt == ND - 1),
                    )
                    if dt % 4 == 3:
                        yield
                nc.vector.tensor_scalar_add(kt_s[:, cs], pk[:], bk_ap)

                pv = psum.tile([128, 512], f32, tag="acc512", bufs=2,
                               name="pv")
                for dt in range(ND):
                    nc.tensor.matmul(
                        pv[:], wv_s[:, dt, :], xt[:, dt, :],
                        start=(dt == 0), stop=(dt == ND - 1),
                    )
                    if dt % 4 == 3:
                        yield
                vt_tmp = vtpool.tile([128, 512], bf16, tag="vt_tmp")
                nc.vector.tensor_scalar_add(vt_tmp[:], pv[:], bv_ap)
                for s in range(4):
                    ktile = rb * 4 + s
                    vp = psum.tile([128, 128], bf16, tag="acc512", bufs=2,
                                   name="vp")
                    nc.tensor.transpose(
                        vp[:], vt_tmp[:, s * 128:(s + 1) * 128], identb_s[:]
                    )
                    nc.vector.tensor_copy(
                        v_s[:, ktile * 128:(ktile + 1) * 128], vp[:]
                    )
                yield

                if rb < LQ // 512:
                    pq = psum.tile([128, 512], f32, tag="acc512", bufs=2,
                                   name="pq")
                    for dt in range(ND):
                        nc.tensor.matmul(
                            pq[:], wq_s[:, dt, :], xt[:, dt, :],
                            start=(dt == 0), stop=(dt == ND - 1),
                        )
                        if dt % 4 == 3:
                            yield
                    nc.vector.tensor_scalar_add(qt_s[:, cs], pq[:], bq_ap)

            def emit_rb(rb, prefetch_rb=None):
                for _ in emit_rb_gen(rb, prefetch_rb):
                    pass

            def build_pairs(phase, u):
                """Pairs of k-tiles sharing one exp: (kt_a, kt_b, mask_off,
                bkey). mask_off indexes masks_s[:, off:off+1024]."""
                pairs = []
                for kblk, kind in SLOTS[phase]:
                    tiles = []
                    for t in range(8):
                        if kind == "diag":
                            drel = t * 128 - u * 512
                            if drel >= 512:
                                continue
                            midx = drel // 128 if drel >= 0 else None
                            tiles.append((kblk * 8 + t, midx))
                        else:
                            tiles.append((kblk * 8 + t, None))
                    bkey = kind if kind in slot_bias else None
                    # tiles with masks come in runs of consecutive midx
                    i = 0
                    while i < len(tiles):
                        (ta, ma), (tb, mb) = tiles[i], tiles[i + 1]
                        assert (ma is None) == (mb is None)
                        moff = None if ma is None else ma * 512
                        pairs.append((ta, tb, moff, bkey))
                        i += 2
                return pairs

            def emit_attn_u(phase, u, filler=None, nfill=1):
                q0 = phase * BLK + u * 512
                pairs = build_pairs(phase, u)
                n = len(pairs)
                ngroups = n // 2
                ot_acc = psum.tile([128, 512], f32, tag="otacc", bufs=1,
                                   name="ot_acc")
                rs_acc = psum.tile([1, 512], f32, tag="rs", bufs=1,
                                   name="rs_acc")
                ests = [None] * n

                def emit_pair(pi):
                    ta, tb, moff, bkey = pairs[pi]
                    stp = psum.tile([128, 1024], f32, tag="stp", bufs=2,
                                    name="stp")
                    nc.tensor.matmul(
                        stp[:, 0:512],
                        kt_s[:, ta * 128:(ta + 1) * 128],
                        qt_s[:, q0:q0 + 512],
                        start=True, stop=True,
                    )
                    nc.tensor.matmul(
                        stp[:, 512:1024],
                        kt_s[:, tb * 128:(tb + 1) * 128],
                        qt_s[:, q0:q0 + 512],
                        start=True, stop=True,
                    )
                    if moff is not None:
                        nc.vector.tensor_add(
                            stp[:], stp[:], masks_s[:, moff:moff + 1024]
                        )
                    est = epool.tile([128, 1024], bf16, tag="est")
                    nc.scalar.activation(
                        est[:], stp[:],
                        mybir.ActivationFunctionType.Exp,
                        bias=slot_bias[bkey] if bkey else 0.0,
                    )
                    ests[pi] = est

                if filler is not None:
                    next(filler, None)
                emit_pair(0)
                if filler is not None:
                    next(filler, None)
                if n > 1:
                    emit_pair(1)
                for pi in range(n):
                    ta, tb, moff, bkey = pairs[pi]
                    if pi + 2 < n:
                        emit_pair(pi + 2)
                    if filler is not None:
                        for _ in range(nfill):
                            next(filler, None)
                    est = ests[pi]
                    nc.tensor.matmul(
                        ot_acc[:],
                        v_s[:, ta * 128:(ta + 1) * 128],
                        est[:, 0:512],
                        start=(pi == 0), stop=False,
                    )
                    nc.tensor.matmul(
                        ot_acc[:],
                        v_s[:, tb * 128:(tb + 1) * 128],
                        est[:, 512:1024],
                        start=False, stop=(pi == n - 1),
                    )
                    if pi % 2 == 1:
                        g = pi // 2
                        esum = epool.tile([128, 1024], bf16, tag="esum",
                                          name="esum")
                        nc.vector.tensor_add(
                            esum[:], ests[pi - 1][:], est[:]
                        )
                        fold = epool.tile([128, 512], bf16, tag="fold",
                                          name="fold")
                        nc.vector.tensor_add(
                            fold[:], esum[:, 0:512], esum[:, 512:1024]
                        )
                        nc.tensor.matmul(
                            rs_acc[:], ones_s[:], fold[:],
                            start=(g == 0), stop=(g == ngroups - 1),
                        )

                qb = phase * 2 + u
                nc.vector.tensor_copy(
                    ot_s[:, qb * 512:(qb + 1) * 512], ot_acc[:]
                )
                nc.vector.tensor_copy(
                    rs_s[:, qb * 512:(qb + 1) * 512], rs_acc[:]
                )
                if filler is not None:
                    for _ in filler:  # drain unconsumed filler chunks
                        pass

            def outproj_gen(qb, on_act=False, nstores=2):
                """Out-projection for one 512-query block into a bf16 slab.
                Stores go on the gpsimd SWDGE queue so they never wait
                behind input-prefetch WAR stalls on the sync queue. Yields
                per dt chunk. on_act alternates copies onto ACT (only for
                regions where ACT is not running exp)."""
                slab = outpool.tile([128, ND, 512], bf16, tag="oslab",
                                    name="oslab")
                per = ND // nstores
                for dt in range(ND):
                    po = psum.tile([128, 512], f32, tag="acc512", bufs=2,
                                   name="po")
                    nc.tensor.matmul(
                        po[:],
                        wo_s[:, dt * 128:(dt + 1) * 128],
                        ot_s[:, qb * 512:(qb + 1) * 512],
                        start=True, stop=True,
                    )
                    if on_act and dt % 2 == 1:
                        nc.scalar.activation(
                            slab[:, dt, :], po[:],
                            mybir.ActivationFunctionType.Copy,
                        )
                    else:
                        nc.vector.tensor_copy(slab[:, dt, :], po[:])
                    if dt % per == per - 1:
                        s = dt + 1 - per
                        nc.gpsimd.dma_start(
                            out_d.ap()[:, qb, s:dt + 1], slab[:, s:dt + 1]
                        )
                    yield

            def emit_outproj(qb, on_act=False, nstores=2):
                for _ in outproj_gen(qb, on_act, nstores):
                    pass

            def chain(*gens):
                for g in gens:
                    for x in g:
                        yield x

            def interleave(*gens):
                live = list(gens)
                while live:
                    for g in list(live):
                        try:
                            next(g)
                        except StopIteration:
                            live.remove(g)

            # ---- interleaved schedule ----
            # phase 0 needs local kblks 0 (rbs 0,1) and 2 (rbs 4,5) plus
            # Qt[0:1024) (rbs 0,1); phase 1 needs everything.
            emit_rb(0, prefetch_rb=4)
            nc.sync.dma_start(wo_s[:], wo_d.ap())
            emit_rb(1, prefetch_rb=5)
            emit_rb(4, prefetch_rb=2)
            emit_rb(5, prefetch_rb=3)
            emit_attn_u(0, 0, filler=emit_rb_gen(2, prefetch_rb=6))
            emit_attn_u(0, 1, filler=emit_rb_gen(3, prefetch_rb=7))
            emit_rb(6)
            emit_rb(7)
            emit_attn_u(1, 0, filler=chain(outproj_gen(0), outproj_gen(1)),
                        nfill=2)
            emit_attn_u(1, 1, filler=outproj_gen(2), nfill=2)
            emit_outproj(3, on_act=True, nstores=4)
            nc.sync.dma_start(rs_d.ap(), rs_s[:])

    nc.compile()
    return nc


def _get_program():
    if "nc" not in _cached:
        _cached["nc"] = _build_program()
    return _cached["nc"]


def _perm_blocks(i):
    # local order [qA, qB, o1, o2]
    return [0, 3, 1, 2] if i == 0 else [1, 2, 0, 3]


def _repack_w(w):
    # (D, HD) -> [128, ND, 128] with per-partition contiguous lines
    return np.ascontiguousarray(
        w.reshape(ND, 128, HD).transpose(1, 0, 2)
    ).astype(ml_dtypes.bfloat16)


def make_in_maps(x, Wq, bq, Wk, bk, Wv, bv, Wo, bo):
    scale = 1.0 / np.sqrt(np.float32(HD))
    wq_r = _repack_w((Wq * scale).astype(np.float32))
    wk_r = _repack_w(Wk.astype(np.float32))
    wv_r = _repack_w(Wv.astype(np.float32))
    bq_s = (bq * scale).astype(np.float32)
    in_maps = []
    for c in range(8):
        i, b = c % 2, c // 2
        perm = _perm_blocks(i)
        xbT = x[b].T  # (D, L) view
        xT = np.concatenate(
            [xbT[:, p * BLK:(p + 1) * BLK] for p in perm], axis=1
        )
        # (D, L) -> [128, NRB, ND, 512]: xr[p, rb, dt, c] = xT[dt*128+p,
        # rb*512+c]
        xr = np.ascontiguousarray(
            xT.reshape(ND, 128, NRB, 512).transpose(1, 2, 0, 3)
        ).astype(ml_dtypes.bfloat16)
        biases = np.zeros((128, 8), np.float32)
        biases[:, 0] = bq_s
        biases[:, 1] = bk.astype(np.float32)
        biases[:, 2] = bv.astype(np.float32)
        biases[:, 3] = NEG if i == 0 else 0.0   # phase A, slot kblk=2
        biases[:, 4] = 0.0 if i == 0 else NEG   # phase B, slot kblk=3
        in_maps.append({
            "xr": xr,
            "wq": wq_r,
            "wk": wk_r,
            "wv": wv_r,
            "wo": Wo.astype(ml_dtypes.bfloat16),
            "biases": biases,
        })
    return in_maps


def assemble_output(results, bo):
    out = np.empty((B, L, D), np.float32)
    for c in range(8):
        i, b = c % 2, c // 2
        perm = _perm_blocks(i)
        arr = np.asarray(results[c]["outT"], dtype=np.float32)
        # [128, 4, ND, 512] -> (D, LQ)
        outT = arr.transpose(2, 0, 1, 3).reshape(D, LQ)
        outT /= np.asarray(results[c]["rowsums"], dtype=np.float32)
        qA, qB = perm[0], perm[1]
        out[b, qA * BLK:(qA + 1) * BLK, :] = outT[:, 0:BLK].T
        out[b, qB * BLK:(qB + 1) * BLK, :] = outT[:, BLK:2 * BLK].T
    out += bo.astype(np.float32)
    return out


def kernel(x, Wq, bq, Wk, bk, Wv, bv, Wo, bo):
    from concourse.bass_utils import run_bass_kernel_spmd

    nc = _get_program()
    in_maps = make_in_maps(
        np.asarray(x), np.asarray(Wq), np.asarray(bq), np.asarray(Wk),
        np.asarray(bk), np.asarray(Wv), np.asarray(bv), np.asarray(Wo),
        np.asarray(bo),
    )
    res = run_bass_kernel_spmd(nc, in_maps, core_ids=list(range(8)))
    return assemble_output(res.results, np.asarray(bo))


# revision 19
# speedup vs baseline: 1.0599x; 1.0294x over previous
"""Causal attention (B=4, L=4096, D=2048, HD=128) on 8 TRN2 NeuronCores.

Sharding: 8 cores = 4 batches x 2 fold-halves. Core c handles batch b=c//2
and query blocks {i, 3-i} (1024 rows each) where i=c%2 — the "fold" split
balances causal attention work exactly across the two cores of a batch.
Each core recomputes K/V for all 4096 keys of its batch (no collectives).

The on-device program is identical on all cores (SPMD); per-core behavior
comes only from the data: a block-permuted repacked input xr and two
slot-bias vectors that enable/disable the two fold-dependent key blocks
(bias 0 keeps scores, bias -50 drives exp() to ~1e-22, i.e. masks).

v2 changes vs v1:
  - host repacks x and the weights so every DMA has >=4KB contiguous
    per-partition lines (one DMA per 512-col xt tile instead of 4, one
    per weight); output is written bf16 in [128, qb, dt, 512] layout,
    two DMAs per 512-query block instead of 16.
  - score tiles are computed in PAIRS ([128,1024] PSUM spanning 2 banks)
    so one ACT exp call covers 2 k-tiles, amortizing the 352-cycle ACT
    fixed overhead (1147ns/pair vs 2x720ns).
  - the est tree-adds for row-sums moved from gpsimd to DVE on pairs.
  - attention units take a "filler" generator (projection or out-proj
    chunks) and interleave one chunk per score pair so PE never waits
    on the ACT exp pipeline.

Layouts (partition dim first):
  xr      [128, 8, 16, 512]  x[b].T block-permuted: [p, rb, dt, col]
  Qt, Kt  [HD=128, Lq/Lk]    projections, head dim on partitions
  v_s     [k, HD] slabs      natural V per 128-key tile (PE transpose)
  scores  [k=128, 1024]      two k-tiles per PSUM pair tile; exp on ACT
  outT    [128, 4, 16, 512]  bf16 [p, qb, dt, col]; host divides by
                             row-sums, transposes back, adds bo
"""

import numpy as np
import ml_dtypes

B, L, D, HD = 4, 4096, 2048, 128
BLK = 1024            # fold block (4 per batch)
LQ = 2 * BLK          # queries per core
LK = L                # keys per core
ND = D // 128         # 16 d-tiles
NRB = LK // 512       # 8 column blocks for projections
NEG = -50.0           # slot-disable bias (exp(x-50) ~ 0)
MASKVAL = -30000.0    # intra-tile causal mask additive value

_cached = {}


def _build_program():
    import concourse.bass as bass
    import concourse.tile as tile
    from concourse import bacc, mybir
    from concourse.masks import make_identity

    f32 = mybir.dt.float32
    bf16 = mybir.dt.bfloat16
    nc = bacc.Bacc("TRN2", target_bir_lowering=False, debug=False)

    xr_d = nc.dram_tensor("xr", (128, NRB, ND, 512), bf16,
                          kind="ExternalInput")
    wq_d = nc.dram_tensor("wq", (128, ND, 128), bf16, kind="ExternalInput")
    wk_d = nc.dram_tensor("wk", (128, ND, 128), bf16, kind="ExternalInput")
    wv_d = nc.dram_tensor("wv", (128, ND, 128), bf16, kind="ExternalInput")
    wo_d = nc.dram_tensor("wo", (HD, D), bf16, kind="ExternalInput")
    bias_d = nc.dram_tensor("biases", (128, 8), f32, kind="ExternalInput")
    out_d = nc.dram_tensor("outT", (128, 4, ND, 512), bf16,
                           kind="ExternalOutput")
    rs_d = nc.dram_tensor("rowsums", (1, LQ), f32, kind="ExternalOutput")

    # phase -> list of (local_kblk, kind); kind in {"diag", "full", "bA", "bB"}
    SLOTS = {
        0: [(0, "diag"), (2, "bA")],
        1: [(0, "full"), (1, "diag"), (2, "full"), (3, "bB")],
    }

    with tile.TileContext(nc) as tc:
        with (
            tc.tile_pool(name="const", bufs=1) as cpool,
            tc.tile_pool(name="xt", bufs=3) as xtpool,
            tc.tile_pool(name="vt", bufs=3) as vtpool,
            tc.tile_pool(name="expst", bufs=6) as epool,
            tc.tile_pool(name="outsb", bufs=2) as outpool,
            tc.tile_pool(name="psum", bufs=1, space="PSUM") as psum,
        ):
            # ---- persistent SBUF tensors ----
            wq_s = cpool.tile([128, ND, 128], bf16, tag="wq")
            wk_s = cpool.tile([128, ND, 128], bf16, tag="wk")
            wv_s = cpool.tile([128, ND, 128], bf16, tag="wv")
            wo_s = cpool.tile([128, D], bf16, tag="wo")
            bias_s = cpool.tile([128, 8], f32, tag="biases")
            kt_s = cpool.tile([128, LK], bf16, tag="kt")
            qt_s = cpool.tile([128, LQ], bf16, tag="qt")
            v_s = cpool.tile([128, LK], bf16, tag="v")
            ones_s = cpool.tile([128, 1], bf16, tag="ones")
            rs_s = cpool.tile([1, LQ], f32, tag="rs")
            masks_s = cpool.tile([128, 4 * 512], f32, tag="masks")
            ot_s = cpool.tile([128, LQ], bf16, tag="ot")
            identb_s = cpool.tile([128, 128], bf16, tag="identb")

            # first xt block + wk first so PE can start ASAP; xt1 ahead of
            # wv/wq so rb1's K can follow rb0 without a DMA underrun; wo is
            # deferred (not needed until the first out-projection).
            xts = {}
            nc.sync.dma_start(wk_s[:, 0:4], wk_d.ap()[:, 0:4])
            xts[0] = xtpool.tile([128, ND, 512], bf16, tag="xt", name="xt")
            nc.sync.dma_start(xts[0][:, 0:2, :], xr_d.ap()[:, 0, 0:2, :])
            nc.sync.dma_start(wk_s[:, 4:ND], wk_d.ap()[:, 4:ND])
            nc.sync.dma_start(xts[0][:, 2:4, :], xr_d.ap()[:, 0, 2:4, :])
            nc.sync.dma_start(xts[0][:, 4:8, :], xr_d.ap()[:, 0, 4:8, :])
            nc.sync.dma_start(xts[0][:, 8:12, :], xr_d.ap()[:, 0, 8:12, :])
            nc.sync.dma_start(xts[0][:, 12:16, :], xr_d.ap()[:, 0, 12:16, :])
            nc.sync.dma_start(wv_s[:], wv_d.ap())
            nc.sync.dma_start(wq_s[:], wq_d.ap())
            nc.sync.dma_start(bias_s[:], bias_d.ap())
            xts[1] = xtpool.tile([128, ND, 512], bf16, tag="xt", name="xt")
            nc.sync.dma_start(xts[1][:], xr_d.ap()[:, 1])

            make_identity(nc, identb_s[:])
            nc.gpsimd.memset(ones_s[:], 1.0)
            # 4 causal mask tiles for relative offsets delta = 0,128,256,384:
            # keep 0 where q_free >= k_part + delta, else MASKVAL
            nc.gpsimd.memset(masks_s[:], 0.0)
            for m in range(4):
                nc.gpsimd.affine_select(
                    out=masks_s[:, m * 512:(m + 1) * 512],
                    in_=masks_s[:, m * 512:(m + 1) * 512],
                    compare_op=mybir.AluOpType.is_ge,
                    fill=MASKVAL,
                    base=-(m * 128),
                    channel_multiplier=-1,
                    pattern=[[1, 512]],
                )

            bq_ap = bias_s[:, 0:1]
            bk_ap = bias_s[:, 1:2]
            bv_ap = bias_s[:, 2:3]
            slot_bias = {"bA": bias_s[:, 3:4], "bB": bias_s[:, 4:5]}

            def prefetch(rb):
                xts[rb] = xtpool.tile([128, ND, 512], bf16, tag="xt",
                                      name="xt")
                nc.sync.dma_start(xts[rb][:], xr_d.ap()[:, rb])

            def emit_rb_gen(rb, prefetch_rb=None):
                """Projections for one 512-wide column block of xr.
                Yields between ~1us chunks so it can fill attention gaps."""
                xt = xts.pop(rb)
                if prefetch_rb is not None:
                    prefetch(prefetch_rb)
                cs = slice(rb * 512, (rb + 1) * 512)

                pk = psum.tile([128, 512], f32, tag="acc512", bufs=2,
                               name="pk")
                for dt in range(ND):
                    nc.tensor.matmul(
                        pk[:], wk_s[:, dt, :], xt[:, dt, :],
                        start=(dt == 0), stop=(dt == ND - 1),
                    )
                    if dt % 4 == 3:
                        yield
                nc.vector.tensor_scalar_add(kt_s[:, cs], pk[:], bk_ap)

                pv = psum.tile([128, 512], f32, tag="acc512", bufs=2,
                               name="pv")
                for dt in range(ND):
                    nc.tensor.matmul(
                        pv[:], wv_s[:, dt, :], xt[:, dt, :],
                        start=(dt == 0), stop=(dt == ND - 1),
                    )
                    if dt % 4 == 3:
                        yield
                vt_tmp = vtpool.tile([128, 512], bf16, tag="vt_tmp")
                nc.vector.tensor_scalar_add(vt_tmp[:], pv[:], bv_ap)
                for s in range(4):
                    ktile = rb * 4 + s
                    vp = psum.tile([128, 128], bf16, tag="acc512", bufs=2,
                                   name="vp")
                    nc.tensor.transpose(
                        vp[:], vt_tmp[:, s * 128:(s + 1) * 128], identb_s[:]
                    )
                    nc.vector.tensor_copy(
                        v_s[:, ktile * 128:(ktile + 1) * 128], vp[:]
                    )
                yield

                if rb < LQ // 512:
                    pq = psum.tile([128, 512], f32, tag="acc512", bufs=2,
                                   name="pq")
                    for dt in range(ND):
                        nc.tensor.matmul(
                            pq[:], wq_s[:, dt, :], xt[:, dt, :],
                            start=(dt == 0), stop=(dt == ND - 1),
                        )
                        if dt % 4 == 3:
                            yield
                    nc.vector.tensor_scalar_add(qt_s[:, cs], pq[:], bq_ap)

            def emit_rb(rb, prefetch_rb=None):
                for _ in emit_rb_gen(rb, prefetch_rb):
                    pass

            def build_pairs(phase, u):
                """Pairs of k-tiles sharing one exp: (kt_a, kt_b, mask_off,
                bkey). mask_off indexes masks_s[:, off:off+1024]."""
                pairs = []
                for kblk, kind in SLOTS[phase]:
                    tiles = []
                    for t in range(8):
                        if kind == "diag":
                            drel = t * 128 - u * 512
                            if drel >= 512:
                                continue
                            midx = drel // 128 if drel >= 0 else None
                            tiles.append((kblk * 8 + t, midx))
                        else:
                            tiles.append((kblk * 8 + t, None))
                    bkey = kind if kind in slot_bias else None
                    # tiles with masks come in runs of consecutive midx
                    i = 0
                    while i < len(tiles):
                        (ta, ma), (tb, mb) = tiles[i], tiles[i + 1]
                        assert (ma is None) == (mb is None)
                        moff = None if ma is None else ma * 512
                        pairs.append((ta, tb, moff, bkey))
                        i += 2
                return pairs

            def emit_attn_u(phase, u, filler=None, nfill=1):
                q0 = phase * BLK + u * 512
                pairs = build_pairs(phase, u)
                n = len(pairs)
                ngroups = n // 2
                ot_acc = psum.tile([128, 512], f32, tag="otacc", bufs=1,
                                   name="ot_acc")
                rs_acc = psum.tile([1, 512], f32, tag="rs", bufs=1,
                                   name="rs_acc")
                ests = [None] * n

                def emit_pair(pi):
                    ta, tb, moff, bkey = pairs[pi]
                    stp = psum.tile([128, 1024], f32, tag="stp", bufs=2,
                                    name="stp")
                    nc.tensor.matmul(
                        stp[:, 0:512],
                        kt_s[:, ta * 128:(ta + 1) * 128],
                        qt_s[:, q0:q0 + 512],
                        start=True, stop=True,
                    )
                    nc.tensor.matmul(
                        stp[:, 512:1024],
                        kt_s[:, tb * 128:(tb + 1) * 128],
                        qt_s[:, q0:q0 + 512],
                        start=True, stop=True,
                    )
                    if moff is not None:
                        nc.vector.tensor_add(
                            stp[:], stp[:], masks_s[:, moff:moff + 1024]
                        )
                    est = epool.tile([128, 1024], bf16, tag="est")
                    nc.scalar.activation(
                        est[:], stp[:],
                        mybir.ActivationFunctionType.Exp,
                        bias=slot_bias[bkey] if bkey else 0.0,
                    )
                    ests[pi] = est

                emit_pair(0)
                if n > 1:
                    emit_pair(1)
                for pi in range(n):
                    ta, tb, moff, bkey = pairs[pi]
                    if pi + 2 < n:
                        emit_pair(pi + 2)
                    if filler is not None:
                        for _ in range(nfill):
                            next(filler, None)
                    est = ests[pi]
                    nc.tensor.matmul(
                        ot_acc[:],
                        v_s[:, ta * 128:(ta + 1) * 128],
                        est[:, 0:512],
                        start=(pi == 0), stop=False,
                    )
                    nc.tensor.matmul(
                        ot_acc[:],
                        v_s[:, tb * 128:(tb + 1) * 128],
                        est[:, 512:1024],
                        start=False, stop=(pi == n - 1),
                    )
                    if pi % 2 == 1:
                        g = pi // 2
                        esum = epool.tile([128, 1024], bf16, tag="esum",
                                          name="esum")
                        nc.vector.tensor_add(
                            esum[:], ests[pi - 1][:], est[:]
                        )
                        fold = epool.tile([128, 512], bf16, tag="fold",
                                          name="fold")
                        nc.vector.tensor_add(
                            fold[:], esum[:, 0:512], esum[:, 512:1024]
                        )
                        nc.tensor.matmul(
                            rs_acc[:], ones_s[:], fold[:],
                            start=(g == 0), stop=(g == ngroups - 1),
                        )

                qb = phase * 2 + u
                nc.vector.tensor_copy(
                    ot_s[:, qb * 512:(qb + 1) * 512], ot_acc[:]
                )
                nc.vector.tensor_copy(
                    rs_s[:, qb * 512:(qb + 1) * 512], rs_acc[:]
                )
                if filler is not None:
                    for _ in filler:  # drain unconsumed filler chunks
                        pass

            def outproj_gen(qb, on_act=False, nstores=2):
                """Out-projection for one 512-query block into a bf16 slab.
                Stores go on the gpsimd SWDGE queue so they never wait
                behind input-prefetch WAR stalls on the sync queue. Yields
                per dt chunk. on_act alternates copies onto ACT (only for
                regions where ACT is not running exp)."""
                slab = outpool.tile([128, ND, 512], bf16, tag="oslab",
                                    name="oslab")
                per = ND // nstores
                for dt in range(ND):
                    po = psum.tile([128, 512], f32, tag="acc512", bufs=2,
                                   name="po")
                    nc.tensor.matmul(
                        po[:],
                        wo_s[:, dt * 128:(dt + 1) * 128],
                        ot_s[:, qb * 512:(qb + 1) * 512],
                        start=True, stop=True,
                    )
                    if on_act and dt % 2 == 1:
                        nc.scalar.activation(
                            slab[:, dt, :], po[:],
                            mybir.ActivationFunctionType.Copy,
                        )
                    else:
                        nc.vector.tensor_copy(slab[:, dt, :], po[:])
                    if dt % per == per - 1:
                        s = dt + 1 - per
                        nc.gpsimd.dma_start(
                            out_d.ap()[:, qb, s:dt + 1], slab[:, s:dt + 1]
                        )
                    yield

            def emit_outproj(qb, on_act=False, nstores=2):
                for _ in outproj_gen(qb, on_act, nstores):
                    pass

            def chain(*gens):
                for g in gens:
                    for x in g:
                        yield x

            def interleave(*gens):
                live = list(gens)
                while live:
                    for g in list(live):
                        try:
                            next(g)
                        except StopIteration:
                            live.remove(g)

            # ---- interleaved schedule ----
            # phase 0 needs local kblks 0 (rbs 0,1) and 2 (rbs 4,5) plus
            # Qt[0:1024) (rbs 0,1); phase 1 needs everything.
            emit_rb(0, prefetch_rb=4)
            nc.sync.dma_start(wo_s[:], wo_d.ap())
            emit_rb(1, prefetch_rb=5)
            emit_rb(4, prefetch_rb=2)
            emit_rb(5, prefetch_rb=3)
            emit_attn_u(0, 0, filler=emit_rb_gen(2, prefetch_rb=6))
            emit_attn_u(0, 1, filler=emit_rb_gen(3, prefetch_rb=7))
            emit_rb(6)
            emit_rb(7)
            emit_attn_u(1, 0, filler=chain(outproj_gen(0), outproj_gen(1)),
                        nfill=2)
            emit_attn_u(1, 1, filler=outproj_gen(2), nfill=2)
            emit_outproj(3, on_act=True, nstores=4)
            nc.sync.dma_start(rs_d.ap(), rs_s[:])

    nc.compile()
    return nc


def _get_program():
    if "nc" not in _cached:
        _cached["nc"] = _build_program()
    return _cached["nc"]


def _perm_blocks(i):
    # local order [qA, qB, o1, o2]
    return [0, 3, 1, 2] if i == 0 else [1, 2, 0, 3]


def _repack_w(w):
    # (D, HD) -> [128, ND, 128] with per-partition contiguous lines
    return np.ascontiguousarray(
        w.reshape(ND, 128, HD).transpose(1, 0, 2)
    ).astype(ml_dtypes.bfloat16)


def make_in_maps(x, Wq, bq, Wk, bk, Wv, bv, Wo, bo):
    scale = 1.0 / np.sqrt(np.float32(HD))
    wq_r = _repack_w((Wq * scale).astype(np.float32))
    wk_r = _repack_w(Wk.astype(np.float32))
    wv_r = _repack_w(Wv.astype(np.float32))
    bq_s = (bq * scale).astype(np.float32)
    in_maps = []
    for c in range(8):
        i, b = c % 2, c // 2
        perm = _perm_blocks(i)
        xbT = x[b].T  # (D, L) view
        xT = np.concatenate(
            [xbT[:, p * BLK:(p + 1) * BLK] for p in perm], axis=1
        )
        # (D, L) -> [128, NRB, ND, 512]: xr[p, rb, dt, c] = xT[dt*128+p,
        # rb*512+c]
        xr = np.ascontiguousarray(
            xT.reshape(ND, 128, NRB, 512).transpose(1, 2, 0, 3)
        ).astype(ml_dtypes.bfloat16)
        biases = np.zeros((128, 8), np.float32)
        biases[:, 0] = bq_s
        biases[:, 1] = bk.astype(np.float32)
        biases[:, 2] = bv.astype(np.float32)
        biases[:, 3] = NEG if i == 0 else 0.0   # phase A, slot kblk=2
        biases[:, 4] = 0.0 if i == 0 else NEG   # phase B, slot kblk=3
        in_maps.append({
            "xr": xr,
            "wq": wq_r,
            "wk": wk_r,
            "wv": wv_r,
            "wo": Wo.astype(ml_dtypes.bfloat16),
            "biases": biases,
        })
    return in_maps


def assemble_output(results, bo):
    out = np.empty((B, L, D), np.float32)
    for c in range(8):
        i, b = c % 2, c // 2
        perm = _perm_blocks(i)
        arr = np.asarray(results[c]["outT"], dtype=np.float32)
        # [128, 4, ND, 512] -> (D, LQ)
        outT = arr.transpose(2, 0, 1, 3).reshape(D, LQ)
        outT /= np.asarray(results[c]["rowsums"], dtype=np.float32)
        qA, qB = perm[0], perm[1]
        out[b, qA * BLK:(qA + 1) * BLK, :] = outT[:, 0:BLK].T
        out[b, qB * BLK:(qB + 1) * BLK, :] = outT[:, BLK:2 * BLK].T
    out += bo.astype(np.float32)
    return out


def kernel(x, Wq, bq, Wk, bk, Wv, bv, Wo, bo):
    from concourse.bass_utils import run_bass_kernel_spmd

    nc = _get_program()
    in_maps = make_in_maps(
        np.asarray(x), np.asarray(Wq), np.asarray(bq), np.asarray(Wk),
        np.asarray(bk), np.asarray(Wv), np.asarray(bv), np.asarray(Wo),
        np.asarray(bo),
    )
    res = run_bass_kernel_spmd(nc, in_maps, core_ids=list(range(8)))
    return assemble_output(res.results, np.asarray(bo))


# revision 26
# speedup vs baseline: 1.1093x; 1.0466x over previous
"""Causal attention (B=4, L=4096, D=2048, HD=128) on 8 TRN2 NeuronCores.

Sharding: 8 cores = 4 batches x 2 fold-halves. Core c handles batch b=c//2
and query blocks {i, 3-i} (1024 rows each) where i=c%2 — the "fold" split
balances causal attention work exactly across the two cores of a batch.
Each core recomputes K/V for all 4096 keys of its batch (no collectives).

The on-device program is identical on all cores (SPMD); per-core behavior
comes only from the data: a block-permuted repacked input xr and two
slot-bias vectors that enable/disable the two fold-dependent key blocks
(bias 0 keeps scores, bias -50 drives exp() to ~1e-22, i.e. masks).

v2 changes vs v1:
  - host repacks x and the weights so every DMA has >=4KB contiguous
    per-partition lines (one DMA per 512-col xt tile instead of 4, one
    per weight); output is written bf16 in [128, qb, dt, 512] layout,
    two DMAs per 512-query block instead of 16.
  - score tiles are computed in PAIRS ([128,1024] PSUM spanning 2 banks)
    so one ACT exp call covers 2 k-tiles, amortizing the 352-cycle ACT
    fixed overhead (1147ns/pair vs 2x720ns).
  - the est tree-adds for row-sums moved from gpsimd to DVE on pairs.
  - attention units take a "filler" generator (projection or out-proj
    chunks) and interleave one chunk per score pair so PE never waits
    on the ACT exp pipeline.

Layouts (partition dim first):
  xr      [128, 8, 16, 512]  x[b].T block-permuted: [p, rb, dt, col]
  Qt, Kt  [HD=128, Lq/Lk]    projections, head dim on partitions
  v_s     [k, HD] slabs      natural V per 128-key tile (PE transpose)
  scores  [k=128, 1024]      two k-tiles per PSUM pair tile; exp on ACT
  outT    [128, 4, 16, 512]  bf16 [p, qb, dt, col]; host divides by
                             row-sums, transposes back, adds bo
"""

import numpy as np
import ml_dtypes

B, L, D, HD = 4, 4096, 2048, 128
BLK = 1024            # fold block (4 per batch)
LQ = 2 * BLK          # queries per core
LK = L                # keys per core
ND = D // 128         # 16 d-tiles
NRB = LK // 512       # 8 column blocks for projections
NEG = -50.0           # slot-disable bias (exp(x-50) ~ 0)
MASKVAL = -30000.0    # intra-tile causal mask additive value

_cached = {}


def _build_program():
    import concourse.bass as bass
    import concourse.tile as tile
    from concourse import bacc, mybir
    from concourse.masks import make_identity

    f32 = mybir.dt.float32
    bf16 = mybir.dt.bfloat16
    nc = bacc.Bacc("TRN2", target_bir_lowering=False, debug=False)

    xr_d = nc.dram_tensor("xr", (128, NRB, ND, 512), bf16,
                          kind="ExternalInput")
    wq_d = nc.dram_tensor("wq", (128, ND, 128), bf16, kind="ExternalInput")
    wk_d = nc.dram_tensor("wk", (128, ND, 128), bf16, kind="ExternalInput")
    wv_d = nc.dram_tensor("wv", (128, ND, 128), bf16, kind="ExternalInput")
    wo_d = nc.dram_tensor("wo", (HD, D), bf16, kind="ExternalInput")
    bias_d = nc.dram_tensor("biases", (128, 8), f32, kind="ExternalInput")
    out_d = nc.dram_tensor("outT", (128, 4, ND, 512), bf16,
                           kind="ExternalOutput")
    rs_d = nc.dram_tensor("rowsums", (1, LQ), f32, kind="ExternalOutput")

    # phase -> list of (local_kblk, kind); kind in {"diag", "full", "bA", "bB"}
    SLOTS = {
        0: [(0, "diag"), (2, "bA")],
        1: [(0, "full"), (1, "diag"), (2, "full"), (3, "bB")],
    }

    with tile.TileContext(nc) as tc:
        with (
            tc.tile_pool(name="const", bufs=1) as cpool,
            tc.tile_pool(name="xt", bufs=3) as xtpool,
            tc.tile_pool(name="vt", bufs=3) as vtpool,
            tc.tile_pool(name="expst", bufs=6) as epool,
            tc.tile_pool(name="outsb", bufs=2) as outpool,
            tc.tile_pool(name="psum", bufs=1, space="PSUM") as psum,
        ):
            # ---- persistent SBUF tensors ----
            wq_s = cpool.tile([128, ND, 128], bf16, tag="wq")
            wk_s = cpool.tile([128, ND, 128], bf16, tag="wk")
            wv_s = cpool.tile([128, ND, 128], bf16, tag="wv")
            wo_s = cpool.tile([128, D], bf16, tag="wo")
            bias_s = cpool.tile([128, 8], f32, tag="biases")
            kt_s = cpool.tile([128, LK], bf16, tag="kt")
            qt_s = cpool.tile([128, LQ], bf16, tag="qt")
            v_s = cpool.tile([128, LK], bf16, tag="v")
            ones_s = cpool.tile([128, 1], bf16, tag="ones")
            rs_s = cpool.tile([1, LQ], f32, tag="rs")
            masks_s = cpool.tile([128, 4 * 512], bf16, tag="masks")
            ot_s = cpool.tile([128, LQ], bf16, tag="ot")
            identb_s = cpool.tile([128, 128], bf16, tag="identb")

            # first xt block + wk first so PE can start ASAP; xt1 ahead of
            # wv/wq so rb1's K can follow rb0 without a DMA underrun; wo is
            # deferred (not needed until the first out-projection).
            xts = {}
            nc.sync.dma_start(wk_s[:], wk_d.ap())
            xts[0] = xtpool.tile([128, ND, 512], bf16, tag="xt", name="xt")
            for ch in range(4):
                nc.sync.dma_start(
                    xts[0][:, ch * 4:(ch + 1) * 4, :],
                    xr_d.ap()[:, 0, ch * 4:(ch + 1) * 4, :],
                )
            nc.sync.dma_start(wv_s[:], wv_d.ap())
            nc.sync.dma_start(wq_s[:], wq_d.ap())
            nc.sync.dma_start(bias_s[:], bias_d.ap())
            xts[1] = xtpool.tile([128, ND, 512], bf16, tag="xt", name="xt")
            nc.sync.dma_start(xts[1][:], xr_d.ap()[:, 1])

            make_identity(nc, identb_s[:])
            nc.gpsimd.memset(ones_s[:], 1.0)
            # preload the ACT exp table during the DMA-bound head (the first
            # real exp would otherwise eat a ~1.3us ACT_TABLE_LOAD stall
            # mid-attention)
            warm = epool.tile([128, 1], bf16, tag="warm", name="warm")
            nc.scalar.activation(
                warm[:], ones_s[:], mybir.ActivationFunctionType.Exp,
                bias=0.0,
            )
            # 4 causal mask tiles for relative offsets delta = 0,128,256,384:
            # keep 0 where q_free >= k_part + delta, else MASKVAL
            nc.gpsimd.memset(masks_s[:], 0.0)
            for m in range(4):
                nc.gpsimd.affine_select(
                    out=masks_s[:, m * 512:(m + 1) * 512],
                    in_=masks_s[:, m * 512:(m + 1) * 512],
                    compare_op=mybir.AluOpType.is_ge,
                    fill=MASKVAL,
                    base=-(m * 128),
                    channel_multiplier=-1,
                    pattern=[[1, 512]],
                )

            bq_ap = bias_s[:, 0:1]
            bk_ap = bias_s[:, 1:2]
            bv_ap = bias_s[:, 2:3]
            slot_bias = {"bA": bias_s[:, 3:4], "bB": bias_s[:, 4:5]}

            def prefetch(rb):
                xts[rb] = xtpool.tile([128, ND, 512], bf16, tag="xt",
                                      name="xt")
                nc.sync.dma_start(xts[rb][:], xr_d.ap()[:, rb])

            def emit_rb_gen(rb, prefetch_rb=None):
                """Projections for one 512-wide column block of xr.
                Yields between ~1us chunks so it can fill attention gaps."""
                xt = xts.pop(rb)
                if prefetch_rb is not None:
                    prefetch(prefetch_rb)
                cs = slice(rb * 512, (rb + 1) * 512)

                pk = psum.tile([128, 512], f32, tag="acc512", bufs=2,
                               name="pk")
                for dt in range(ND):
                    nc.tensor.matmul(
                        pk[:], wk_s[:, dt, :], xt[:, dt, :],
                        start=(dt == 0), stop=(dt == ND - 1),
                    )
                    if dt % 4 == 3:
                        yield
                nc.vector.tensor_scalar_add(kt_s[:, cs], pk[:], bk_ap)

                pv = psum.tile([128, 512], f32, tag="acc512", bufs=2,
                               name="pv")
                for dt in range(ND):
                    nc.tensor.matmul(
                        pv[:], wv_s[:, dt, :], xt[:, dt, :],
                        start=(dt == 0), stop=(dt == ND - 1),
                    )
                    if dt % 4 == 3:
                        yield
                vt_tmp = vtpool.tile([128, 512], bf16, tag="vt_tmp")
                nc.vector.tensor_scalar_add(vt_tmp[:], pv[:], bv_ap)
                for s in range(4):
                    ktile = rb * 4 + s
                    vp = psum.tile([128, 128], bf16, tag="acc512", bufs=2,
                                   name="vp")
                    nc.tensor.transpose(
                        vp[:], vt_tmp[:, s * 128:(s + 1) * 128], identb_s[:]
                    )
                    nc.vector.tensor_copy(
                        v_s[:, ktile * 128:(ktile + 1) * 128], vp[:]
                    )
                yield

                if rb < LQ // 512:
                    pq = psum.tile([128, 512], f32, tag="acc512", bufs=2,
                                   name="pq")
                    for dt in range(ND):
                        nc.tensor.matmul(
                            pq[:], wq_s[:, dt, :], xt[:, dt, :],
                            start=(dt == 0), stop=(dt == ND - 1),
                        )
                        if dt % 4 == 3:
                            yield
                    nc.vector.tensor_scalar_add(qt_s[:, cs], pq[:], bq_ap)

            def emit_rb(rb, prefetch_rb=None):
                for _ in emit_rb_gen(rb, prefetch_rb):
                    pass

            def build_pairs(phase, u):
                """Pairs of k-tiles sharing one exp: (kt_a, kt_b, mask_off,
                bkey). mask_off indexes masks_s[:, off:off+1024]."""
                pairs = []
                for kblk, kind in SLOTS[phase]:
                    tiles = []
                    for t in range(8):
                        if kind == "diag":
                            drel = t * 128 - u * 512
                            if drel >= 512:
                                continue
                            midx = drel // 128 if drel >= 0 else None
                            tiles.append((kblk * 8 + t, midx))
                        else:
                            tiles.append((kblk * 8 + t, None))
                    bkey = kind if kind in slot_bias else None
                    # tiles with masks come in runs of consecutive midx
                    i = 0
                    while i < len(tiles):
                        (ta, ma), (tb, mb) = tiles[i], tiles[i + 1]
                        assert (ma is None) == (mb is None)
                        moff = None if ma is None else ma * 512
                        pairs.append((ta, tb, moff, bkey))
                        i += 2
                return pairs

            def emit_attn_u(phase, u, filler=None, nfill=1):
                q0 = phase * BLK + u * 512
                pairs = build_pairs(phase, u)
                n = len(pairs)
                ngroups = n // 2
                ot_acc = psum.tile([128, 512], f32, tag="otacc", bufs=1,
                                   name="ot_acc")
                rs_acc = psum.tile([1, 512], f32, tag="rs", bufs=1,
                                   name="rs_acc")
                ests = [None] * n

                def emit_pair(pi):
                    ta, tb, moff, bkey = pairs[pi]
                    stp = psum.tile([128, 1024], f32, tag="stp", bufs=2,
                                    name="stp")
                    # causal masks are pre-loaded into PSUM via an identity
                    # matmul and the score matmul accumulates on top — this
                    # keeps the mask off the DVE (a [128,1024] PSUM
                    # tensor_add costs ~1.2us and serializes score->exp)
                    for h, kt in ((0, ta), (1, tb)):
                        hs = slice(h * 512, (h + 1) * 512)
                        if moff is not None:
                            nc.tensor.matmul(
                                stp[:, hs], identb_s[:],
                                masks_s[:, moff + h * 512:
                                        moff + (h + 1) * 512],
                                start=True, stop=False,
                            )
                        nc.tensor.matmul(
                            stp[:, hs],
                            kt_s[:, kt * 128:(kt + 1) * 128],
                            qt_s[:, q0:q0 + 512],
                            start=(moff is None), stop=True,
                        )
                    est = epool.tile([128, 1024], bf16, tag="est")
                    nc.scalar.activation(
                        est[:], stp[:],
                        mybir.ActivationFunctionType.Exp,
                        bias=slot_bias[bkey] if bkey else 0.0,
                    )
                    ests[pi] = est

                emit_pair(0)
                if n > 1:
                    emit_pair(1)
                for pi in range(n):
                    ta, tb, moff, bkey = pairs[pi]
                    if pi + 2 < n:
                        emit_pair(pi + 2)
                    if filler is not None:
                        for _ in range(nfill):
                            next(filler, None)
                    est = ests[pi]
                    nc.tensor.matmul(
                        ot_acc[:],
                        v_s[:, ta * 128:(ta + 1) * 128],
                        est[:, 0:512],
                        start=(pi == 0), stop=False,
                    )
                    nc.tensor.matmul(
                        ot_acc[:],
                        v_s[:, tb * 128:(tb + 1) * 128],
                        est[:, 512:1024],
                        start=False, stop=(pi == n - 1),
                    )
                    if pi % 2 == 1:
                        g = pi // 2
                        esum = epool.tile([128, 1024], bf16, tag="esum",
                                          name="esum")
                        nc.vector.tensor_add(
                            esum[:], ests[pi - 1][:], est[:]
                        )
                        fold = epool.tile([128, 512], bf16, tag="fold",
                                          name="fold")
                        nc.vector.tensor_add(
                            fold[:], esum[:, 0:512], esum[:, 512:1024]
                        )
                        nc.tensor.matmul(
                            rs_acc[:], ones_s[:], fold[:],
                            start=(g == 0), stop=(g == ngroups - 1),
                        )

                qb = phase * 2 + u
                nc.vector.tensor_copy(
                    ot_s[:, qb * 512:(qb + 1) * 512], ot_acc[:]
                )
                nc.vector.tensor_copy(
                    rs_s[:, qb * 512:(qb + 1) * 512], rs_acc[:]
                )
                if filler is not None:
                    for _ in filler:  # drain unconsumed filler chunks
                        pass

            def outproj_gen(qb, on_act=False, nstores=2, use_stp=False):
                """Out-projection for one 512-query block into a bf16 slab.
                Stores go on the gpsimd SWDGE queue so they never wait
                behind input-prefetch WAR stalls on the sync queue. Yields
                per dt chunk. on_act alternates copies onto ACT (only for
                regions where ACT is not running exp). use_stp borrows the
                stp PSUM banks for a 4-deep drain pipeline — only safe
                after all attention units are done."""
                slab = outpool.tile([128, ND, 512], bf16, tag="oslab",
                                    name="oslab")
                per = ND // nstores
                pop = None
                for dt in range(ND):
                    if use_stp:
                        if dt % 2 == 0:
                            pop = psum.tile([128, 1024], f32, tag="stp",
                                            bufs=2, name="po")
                        po = pop[:, (dt % 2) * 512:(dt % 2) * 512 + 512]
                    else:
                        po = psum.tile([128, 512], f32, tag="acc512",
                                       bufs=2, name="po")[:]
                    nc.tensor.matmul(
                        po,
                        wo_s[:, dt * 128:(dt + 1) * 128],
                        ot_s[:, qb * 512:(qb + 1) * 512],
                        start=True, stop=True,
                    )
                    if on_act and dt % 2 == 1:
                        nc.scalar.activation(
                            slab[:, dt, :], po,
                            mybir.ActivationFunctionType.Copy,
                        )
                    else:
                        nc.vector.tensor_copy(slab[:, dt, :], po)
                    if dt % per == per - 1:
                        s = dt + 1 - per
                        nc.gpsimd.dma_start(
                            out_d.ap()[:, qb, s:dt + 1], slab[:, s:dt + 1]
                        )
                    yield

            def emit_outproj(qb, on_act=False, nstores=2, use_stp=False):
                for _ in outproj_gen(qb, on_act, nstores, use_stp):
                    pass

            def chain(*gens):
                for g in gens:
                    for x in g:
                        yield x

            def interleave(*gens):
                live = list(gens)
                while live:
                    for g in list(live):
                        try:
                            next(g)
                        except StopIteration:
                            live.remove(g)

            # ---- interleaved schedule ----
            # phase 0 needs local kblks 0 (rbs 0,1) and 2 (rbs 4,5) plus
            # Qt[0:1024) (rbs 0,1); phase 1 needs everything.
            emit_rb(0, prefetch_rb=4)
            nc.sync.dma_start(wo_s[:], wo_d.ap())
            emit_rb(1, prefetch_rb=5)
            emit_rb(4, prefetch_rb=2)
            emit_rb(5, prefetch_rb=3)
            emit_attn_u(0, 0, filler=emit_rb_gen(2, prefetch_rb=6))
            emit_attn_u(0, 1, filler=emit_rb_gen(3, prefetch_rb=7))
            # out-proj of qb0 rides along rb6/rb7: its matmuls interleave
            # with projection matmuls on PE while its PSUM drains use the
            # exp-free DVE/ACT window; attn(1,*) then only carries one
            # out-proj each at 1 chunk/pair so DVE doesn't saturate
            interleave(chain(emit_rb_gen(6), emit_rb_gen(7)),
                       outproj_gen(0, on_act=True))
            emit_attn_u(1, 0, filler=outproj_gen(1), nfill=1)
            emit_attn_u(1, 1, filler=outproj_gen(2), nfill=1)
            emit_outproj(3, on_act=True, nstores=4, use_stp=True)
            nc.sync.dma_start(rs_d.ap(), rs_s[:])

    nc.compile()
    return nc


def _get_program():
    if "nc" not in _cached:
        _cached["nc"] = _build_program()
    return _cached["nc"]


def _perm_blocks(i):
    # local order [qA, qB, o1, o2]
    return [0, 3, 1, 2] if i == 0 else [1, 2, 0, 3]


def _repack_w(w):
    # (D, HD) -> [128, ND, 128] with per-partition contiguous lines
    return np.ascontiguousarray(
        w.reshape(ND, 128, HD).transpose(1, 0, 2)
    ).astype(ml_dtypes.bfloat16)


def make_in_maps(x, Wq, bq, Wk, bk, Wv, bv, Wo, bo):
    scale = 1.0 / np.sqrt(np.float32(HD))
    wq_r = _repack_w((Wq * scale).astype(np.float32))
    wk_r = _repack_w(Wk.astype(np.float32))
    wv_r = _repack_w(Wv.astype(np.float32))
    bq_s = (bq * scale).astype(np.float32)
    in_maps = []
    for c in range(8):
        i, b = c % 2, c // 2
        perm = _perm_blocks(i)
        xbT = x[b].T  # (D, L) view
        xT = np.concatenate(
            [xbT[:, p * BLK:(p + 1) * BLK] for p in perm], axis=1
        )
        # (D, L) -> [128, NRB, ND, 512]: xr[p, rb, dt, c] = xT[dt*128+p,
        # rb*512+c]
        xr = np.ascontiguousarray(
            xT.reshape(ND, 128, NRB, 512).transpose(1, 2, 0, 3)
        ).astype(ml_dtypes.bfloat16)
        biases = np.zeros((128, 8), np.float32)
        biases[:, 0] = bq_s
        biases[:, 1] = bk.astype(np.float32)
        biases[:, 2] = bv.astype(np.float32)
        biases[:, 3] = NEG if i == 0 else 0.0   # phase A, slot kblk=2
        biases[:, 4] = 0.0 if i == 0 else NEG   # phase B, slot kblk=3
        in_maps.append({
            "xr": xr,
            "wq": wq_r,
            "wk": wk_r,
            "wv": wv_r,
            "wo": Wo.astype(ml_dtypes.bfloat16),
            "biases": biases,
        })
    return in_maps


def assemble_output(results, bo):
    out = np.empty((B, L, D), np.float32)
    for c in range(8):
        i, b = c % 2, c // 2
        perm = _perm_blocks(i)
        arr = np.asarray(results[c]["outT"], dtype=np.float32)
        # [128, 4, ND, 512] -> (D, LQ)
        outT = arr.transpose(2, 0, 1, 3).reshape(D, LQ)
        outT /= np.asarray(results[c]["rowsums"], dtype=np.float32)
        qA, qB = perm[0], perm[1]
        out[b, qA * BLK:(qA + 1) * BLK, :] = outT[:, 0:BLK].T
        out[b, qB * BLK:(qB + 1) * BLK, :] = outT[:, BLK:2 * BLK].T
    out += bo.astype(np.float32)
    return out


def kernel(x, Wq, bq, Wk, bk, Wv, bv, Wo, bo):
    from concourse.bass_utils import run_bass_kernel_spmd

    nc = _get_program()
    in_maps = make_in_maps(
        np.asarray(x), np.asarray(Wq), np.asarray(bq), np.asarray(Wk),
        np.asarray(bk), np.asarray(Wv), np.asarray(bv), np.asarray(Wo),
        np.asarray(bo),
    )
    res = run_bass_kernel_spmd(nc, in_maps, core_ids=list(range(8)))
    return assemble_output(res.results, np.asarray(bo))


# revision 30
# speedup vs baseline: 1.1150x; 1.0052x over previous
"""Causal attention (B=4, L=4096, D=2048, HD=128) on 8 TRN2 NeuronCores.

Sharding: 8 cores = 4 batches x 2 fold-halves. Core c handles batch b=c//2
and query blocks {i, 3-i} (1024 rows each) where i=c%2 — the "fold" split
balances causal attention work exactly across the two cores of a batch.
Each core recomputes K/V for all 4096 keys of its batch (no collectives).

The on-device program is identical on all cores (SPMD); per-core behavior
comes only from the data: a block-permuted repacked input xr and two
slot-bias vectors that enable/disable the two fold-dependent key blocks
(bias 0 keeps scores, bias -50 drives exp() to ~1e-22, i.e. masks).

v2 changes vs v1:
  - host repacks x and the weights so every DMA has >=4KB contiguous
    per-partition lines (one DMA per 512-col xt tile instead of 4, one
    per weight); output is written bf16 in [128, qb, dt, 512] layout,
    two DMAs per 512-query block instead of 16.
  - score tiles are computed in PAIRS ([128,1024] PSUM spanning 2 banks)
    so one ACT exp call covers 2 k-tiles, amortizing the 352-cycle ACT
    fixed overhead (1147ns/pair vs 2x720ns).
  - the est tree-adds for row-sums moved from gpsimd to DVE on pairs.
  - attention units take a "filler" generator (projection or out-proj
    chunks) and interleave one chunk per score pair so PE never waits
    on the ACT exp pipeline.

Layouts (partition dim first):
  xr      [128, 8, 16, 512]  x[b].T block-permuted: [p, rb, dt, col]
  Qt, Kt  [HD=128, Lq/Lk]    projections, head dim on partitions
  v_s     [k, HD] slabs      natural V per 128-key tile (PE transpose)
  scores  [k=128, 1024]      two k-tiles per PSUM pair tile; exp on ACT
  outT    [128, 4, 16, 512]  bf16 [p, qb, dt, col]; host divides by
                             row-sums, transposes back, adds bo
"""

import numpy as np
import ml_dtypes

B, L, D, HD = 4, 4096, 2048, 128
BLK = 1024            # fold block (4 per batch)
LQ = 2 * BLK          # queries per core
LK = L                # keys per core
ND = D // 128         # 16 d-tiles
NRB = LK // 512       # 8 column blocks for projections
NEG = -50.0           # slot-disable bias (exp(x-50) ~ 0)
MASKVAL = -30000.0    # intra-tile causal mask additive value

_cached = {}


def _build_program():
    import concourse.bass as bass
    import concourse.tile as tile
    from concourse import bacc, mybir
    from concourse.masks import make_identity

    f32 = mybir.dt.float32
    bf16 = mybir.dt.bfloat16
    nc = bacc.Bacc("TRN2", target_bir_lowering=False, debug=False)

    xr_d = nc.dram_tensor("xr", (128, NRB, ND, 512), bf16,
                          kind="ExternalInput")
    wq_d = nc.dram_tensor("wq", (128, ND, 128), bf16, kind="ExternalInput")
    wk_d = nc.dram_tensor("wk", (128, ND, 128), bf16, kind="ExternalInput")
    wv_d = nc.dram_tensor("wv", (128, ND, 128), bf16, kind="ExternalInput")
    wo_d = nc.dram_tensor("wo", (HD, D), bf16, kind="ExternalInput")
    bias_d = nc.dram_tensor("biases", (128, 8), f32, kind="ExternalInput")
    out_d = nc.dram_tensor("outT", (128, 4, ND, 512), bf16,
                           kind="ExternalOutput")
    rs_d = nc.dram_tensor("rowsums", (1, LQ), f32, kind="ExternalOutput")

    # phase -> list of (local_kblk, kind); kind in {"diag", "full", "bA", "bB"}
    SLOTS = {
        0: [(0, "diag"), (2, "bA")],
        1: [(0, "full"), (1, "diag"), (2, "full"), (3, "bB")],
    }

    with tile.TileContext(nc) as tc:
        with (
            tc.tile_pool(name="const", bufs=1) as cpool,
            tc.tile_pool(name="xt", bufs=3) as xtpool,
            tc.tile_pool(name="vt", bufs=3) as vtpool,
            tc.tile_pool(name="expst", bufs=6) as epool,
            tc.tile_pool(name="outsb", bufs=2) as outpool,
            tc.tile_pool(name="psum", bufs=1, space="PSUM") as psum,
        ):
            # ---- persistent SBUF tensors ----
            wq_s = cpool.tile([128, ND, 128], bf16, tag="wq")
            wk_s = cpool.tile([128, ND, 128], bf16, tag="wk")
            wv_s = cpool.tile([128, ND, 128], bf16, tag="wv")
            wo_s = cpool.tile([128, D], bf16, tag="wo")
            bias_s = cpool.tile([128, 8], f32, tag="biases")
            kt_s = cpool.tile([128, LK], bf16, tag="kt")
            qt_s = cpool.tile([128, LQ], bf16, tag="qt")
            v_s = cpool.tile([128, LK], bf16, tag="v")
            ones_s = cpool.tile([128, 1], bf16, tag="ones")
            rs_s = cpool.tile([1, LQ], f32, tag="rs")
            masks_s = cpool.tile([128, 4 * 512], bf16, tag="masks")
            ot_s = cpool.tile([128, LQ], bf16, tag="ot")
            identb_s = cpool.tile([128, 128], bf16, tag="identb")

            # first xt block + wk first so PE can start ASAP; xt1 ahead of
            # wv/wq so rb1's K can follow rb0 without a DMA underrun; wo is
            # deferred (not needed until the first out-projection).
            xts = {}
            nc.sync.dma_start(wk_s[:], wk_d.ap())
            xts[0] = xtpool.tile([128, ND, 512], bf16, tag="xt", name="xt")
            for ch in range(4):
                nc.sync.dma_start(
                    xts[0][:, ch * 4:(ch + 1) * 4, :],
                    xr_d.ap()[:, 0, ch * 4:(ch + 1) * 4, :],
                )
            nc.sync.dma_start(wv_s[:], wv_d.ap())
            nc.sync.dma_start(wq_s[:], wq_d.ap())
            nc.sync.dma_start(bias_s[:], bias_d.ap())
            xts[1] = xtpool.tile([128, ND, 512], bf16, tag="xt", name="xt")
            nc.sync.dma_start(xts[1][:], xr_d.ap()[:, 1])

            make_identity(nc, identb_s[:])
            nc.gpsimd.memset(ones_s[:], 1.0)
            # preload the ACT exp table during the DMA-bound head (the first
            # real exp would otherwise eat a ~1.3us ACT_TABLE_LOAD stall
            # mid-attention)
            warm = epool.tile([128, 1], bf16, tag="warm", name="warm")
            nc.scalar.activation(
                warm[:], ones_s[:], mybir.ActivationFunctionType.Exp,
                bias=0.0,
            )
            # 4 causal mask tiles for relative offsets delta = 0,128,256,384:
            # keep 0 where q_free >= k_part + delta, else MASKVAL
            nc.gpsimd.memset(masks_s[:], 0.0)
            for m in range(4):
                nc.gpsimd.affine_select(
                    out=masks_s[:, m * 512:(m + 1) * 512],
                    in_=masks_s[:, m * 512:(m + 1) * 512],
                    compare_op=mybir.AluOpType.is_ge,
                    fill=MASKVAL,
                    base=-(m * 128),
                    channel_multiplier=-1,
                    pattern=[[1, 512]],
                )

            bq_ap = bias_s[:, 0:1]
            bk_ap = bias_s[:, 1:2]
            bv_ap = bias_s[:, 2:3]
            slot_bias = {"bA": bias_s[:, 3:4], "bB": bias_s[:, 4:5]}

            def prefetch(rb):
                xts[rb] = xtpool.tile([128, ND, 512], bf16, tag="xt",
                                      name="xt")
                nc.sync.dma_start(xts[rb][:], xr_d.ap()[:, rb])

            def emit_rb_gen(rb, prefetch_rb=None):
                """Projections for one 512-wide column block of xr.
                Yields between ~1us chunks so it can fill attention gaps."""
                xt = xts.pop(rb)
                if prefetch_rb is not None:
                    prefetch(prefetch_rb)
                cs = slice(rb * 512, (rb + 1) * 512)

                pk = psum.tile([128, 512], f32, tag="acc512", bufs=2,
                               name="pk")
                for dt in range(ND):
                    nc.tensor.matmul(
                        pk[:], wk_s[:, dt, :], xt[:, dt, :],
                        start=(dt == 0), stop=(dt == ND - 1),
                    )
                    if dt % 4 == 3:
                        yield
                nc.vector.tensor_scalar_add(kt_s[:, cs], pk[:], bk_ap)

                pv = psum.tile([128, 512], f32, tag="acc512", bufs=2,
                               name="pv")
                for dt in range(ND):
                    nc.tensor.matmul(
                        pv[:], wv_s[:, dt, :], xt[:, dt, :],
                        start=(dt == 0), stop=(dt == ND - 1),
                    )
                    if dt % 4 == 3:
                        yield
                vt_tmp = vtpool.tile([128, 512], bf16, tag="vt_tmp")
                nc.vector.tensor_scalar_add(vt_tmp[:], pv[:], bv_ap)
                for s in range(4):
                    ktile = rb * 4 + s
                    vp = psum.tile([128, 128], bf16, tag="acc512", bufs=2,
                                   name="vp")
                    nc.tensor.transpose(
                        vp[:], vt_tmp[:, s * 128:(s + 1) * 128], identb_s[:]
                    )
                    nc.vector.tensor_copy(
                        v_s[:, ktile * 128:(ktile + 1) * 128], vp[:]
                    )
                yield

                if rb < LQ // 512:
                    pq = psum.tile([128, 512], f32, tag="acc512", bufs=2,
                                   name="pq")
                    for dt in range(ND):
                        nc.tensor.matmul(
                            pq[:], wq_s[:, dt, :], xt[:, dt, :],
                            start=(dt == 0), stop=(dt == ND - 1),
                        )
                        if dt % 4 == 3:
                            yield
                    nc.vector.tensor_scalar_add(qt_s[:, cs], pq[:], bq_ap)

            def emit_rb(rb, prefetch_rb=None):
                for _ in emit_rb_gen(rb, prefetch_rb):
                    pass

            def build_pairs(phase, u):
                """Pairs of k-tiles sharing one exp: (kt_a, kt_b, midx_a,
                bkey). Masked halves use masks_s tile midx_a + h. Diagonal
                pairs go LAST so the first AV matmul (start=True) always
                covers the full query range."""
                pairs = []
                diag_pairs = []
                for kblk, kind in SLOTS[phase]:
                    tiles = []
                    for t in range(8):
                        if kind == "diag":
                            drel = t * 128 - u * 512
                            if drel >= 512:
                                continue
                            midx = drel // 128 if drel >= 0 else None
                            tiles.append((kblk * 8 + t, midx))
                        else:
                            tiles.append((kblk * 8 + t, None))
                    bkey = kind if kind in slot_bias else None
                    dst = diag_pairs if kind == "diag" else pairs
                    # tiles with masks come in runs of consecutive midx
                    i = 0
                    while i < len(tiles):
                        (ta, ma), (tb, mb) = tiles[i], tiles[i + 1]
                        assert (ma is None) == (mb is None)
                        dst.append((ta, tb, ma, bkey))
                        i += 2
                return pairs + diag_pairs

            def emit_attn_u(phase, u, filler=None, nfill=1):
                q0 = phase * BLK + u * 512
                pairs = build_pairs(phase, u)
                n = len(pairs)
                ngroups = n // 2
                ot_acc = psum.tile([128, 512], f32, tag="otacc", bufs=1,
                                   name="ot_acc")
                rs_acc = psum.tile([1, 512], f32, tag="rs", bufs=1,
                                   name="rs_acc")
                ests = [None] * n

                def emit_pair(pi):
                    ta, tb, ma, bkey = pairs[pi]
                    stp = psum.tile([128, 1024], f32, tag="stp", bufs=2,
                                    name="stp")
                    # causal masks are pre-loaded into PSUM via an identity
                    # matmul and the score matmul accumulates on top — this
                    # keeps the mask off the DVE (a [128,1024] PSUM
                    # tensor_add costs ~1.2us and serializes score->exp).
                    # The score matmul then only covers the live query range
                    # (the mask-init already filled the dead zone with
                    # MASKVAL, so exp drives it to 0).
                    for h, kt in ((0, ta), (1, tb)):
                        if ma is not None:
                            m = ma + h
                            nc.tensor.matmul(
                                stp[:, h * 512:(h + 1) * 512], identb_s[:],
                                masks_s[:, m * 512:(m + 1) * 512],
                                start=True, stop=False,
                                skip_group_check=True,
                            )
                            off = m * 128
                            nc.tensor.matmul(
                                stp[:, h * 512 + off:(h + 1) * 512],
                                kt_s[:, kt * 128:(kt + 1) * 128],
                                qt_s[:, q0 + off:q0 + 512],
                                start=False, stop=True,
                                skip_group_check=True,
                            )
                        else:
                            nc.tensor.matmul(
                                stp[:, h * 512:(h + 1) * 512],
                                kt_s[:, kt * 128:(kt + 1) * 128],
                                qt_s[:, q0:q0 + 512],
                                start=True, stop=True,
                            )
                    est = epool.tile([128, 1024], bf16, tag="est")
                    nc.scalar.activation(
                        est[:], stp[:],
                        mybir.ActivationFunctionType.Exp,
                        bias=slot_bias[bkey] if bkey else 0.0,
                    )
                    ests[pi] = est

                emit_pair(0)
                if n > 1:
                    emit_pair(1)
                for pi in range(n):
                    ta, tb, moff, bkey = pairs[pi]
                    if pi + 2 < n:
                        emit_pair(pi + 2)
                    if filler is not None:
                        for _ in range(nfill):
                            next(filler, None)
                    est = ests[pi]
                    nc.tensor.matmul(
                        ot_acc[:],
                        v_s[:, ta * 128:(ta + 1) * 128],
                        est[:, 0:512],
                        start=(pi == 0), stop=False,
                    )
                    nc.tensor.matmul(
                        ot_acc[:],
                        v_s[:, tb * 128:(tb + 1) * 128],
                        est[:, 512:1024],
                        start=False, stop=(pi == n - 1),
                    )
                    if pi % 2 == 1:
                        g = pi // 2
                        esum = epool.tile([128, 1024], bf16, tag="esum",
                                          name="esum")
                        nc.vector.tensor_add(
                            esum[:], ests[pi - 1][:], est[:]
                        )
                        fold = epool.tile([128, 512], bf16, tag="fold",
                                          name="fold")
                        nc.vector.tensor_add(
                            fold[:], esum[:, 0:512], esum[:, 512:1024]
                        )
                        nc.tensor.matmul(
                            rs_acc[:], ones_s[:], fold[:],
                            start=(g == 0), stop=(g == ngroups - 1),
                        )

                qb = phase * 2 + u
                # split the u-end ot drain across DVE and ACT so the next
                # consumer (out-proj matmul) unblocks ~2x sooner
                nc.vector.tensor_copy(
                    ot_s[:, qb * 512:qb * 512 + 256], ot_acc[:, 0:256]
                )
                nc.scalar.activation(
                    ot_s[:, qb * 512 + 256:(qb + 1) * 512],
                    ot_acc[:, 256:512],
                    mybir.ActivationFunctionType.Copy,
                )
                nc.vector.tensor_copy(
                    rs_s[:, qb * 512:(qb + 1) * 512], rs_acc[:]
                )
                if filler is not None:
                    for _ in filler:  # drain unconsumed filler chunks
                        pass

            def outproj_gen(qb, on_act=False, nstores=2, use_stp=False):
                """Out-projection for one 512-query block into a bf16 slab.
                Stores go on the gpsimd SWDGE queue so they never wait
                behind input-prefetch WAR stalls on the sync queue. Yields
                per dt chunk. on_act alternates copies onto ACT (only for
                regions where ACT is not running exp). use_stp borrows the
                stp PSUM banks for a 4-deep drain pipeline — only safe
                after all attention units are done."""
                slab = outpool.tile([128, ND, 512], bf16, tag="oslab",
                                    name="oslab")
                per = ND // nstores
                pop = None
                for dt in range(ND):
                    if use_stp:
                        if dt % 2 == 0:
                            pop = psum.tile([128, 1024], f32, tag="stp",
                                            bufs=2, name="po")
                        po = pop[:, (dt % 2) * 512:(dt % 2) * 512 + 512]
                    else:
                        po = psum.tile([128, 512], f32, tag="acc512",
                                       bufs=2, name="po")[:]
                    nc.tensor.matmul(
                        po,
                        wo_s[:, dt * 128:(dt + 1) * 128],
                        ot_s[:, qb * 512:(qb + 1) * 512],
                        start=True, stop=True,
                    )
                    if on_act and dt % 2 == 1:
                        nc.scalar.activation(
                            slab[:, dt, :], po,
                            mybir.ActivationFunctionType.Copy,
                        )
                    else:
                        nc.vector.tensor_copy(slab[:, dt, :], po)
                    if dt % per == per - 1:
                        s = dt + 1 - per
                        nc.sync.dma_start(
                            out_d.ap()[:, qb, s:dt + 1], slab[:, s:dt + 1]
                        )
                    yield

            def emit_outproj(qb, on_act=False, nstores=2, use_stp=False):
                for _ in outproj_gen(qb, on_act, nstores, use_stp):
                    pass

            def chain(*gens):
                for g in gens:
                    for x in g:
                        yield x

            def interleave(*gens):
                live = list(gens)
                while live:
                    for g in list(live):
                        try:
                            next(g)
                        except StopIteration:
                            live.remove(g)

            # ---- interleaved schedule ----
            # phase 0 needs local kblks 0 (rbs 0,1) and 2 (rbs 4,5) plus
            # Qt[0:1024) (rbs 0,1); phase 1 needs everything.
            emit_rb(0, prefetch_rb=4)
            nc.sync.dma_start(wo_s[:], wo_d.ap())
            emit_rb(1, prefetch_rb=5)
            emit_rb(4, prefetch_rb=2)
            emit_rb(5, prefetch_rb=3)
            emit_attn_u(0, 0, filler=emit_rb_gen(2, prefetch_rb=6))
            emit_attn_u(0, 1, filler=emit_rb_gen(3, prefetch_rb=7))
            # out-proj of qb0 rides along rb6/rb7: its matmuls interleave
            # with projection matmuls on PE while its PSUM drains use the
            # exp-free DVE/ACT window; attn(1,*) then only carries one
            # out-proj each at 1 chunk/pair so DVE doesn't saturate
            interleave(chain(emit_rb_gen(6), emit_rb_gen(7)),
                       outproj_gen(0, on_act=True))
            emit_attn_u(1, 0, filler=outproj_gen(1), nfill=1)
            emit_attn_u(1, 1, filler=outproj_gen(2), nfill=1)
            emit_outproj(3, on_act=True, nstores=4, use_stp=True)
            nc.sync.dma_start(rs_d.ap(), rs_s[:])

    nc.compile()
    return nc


def _get_program():
    if "nc" not in _cached:
        _cached["nc"] = _build_program()
    return _cached["nc"]


def _perm_blocks(i):
    # local order [qA, qB, o1, o2]
    return [0, 3, 1, 2] if i == 0 else [1, 2, 0, 3]


def _repack_w(w):
    # (D, HD) -> [128, ND, 128] with per-partition contiguous lines
    return np.ascontiguousarray(
        w.reshape(ND, 128, HD).transpose(1, 0, 2)
    ).astype(ml_dtypes.bfloat16)


def make_in_maps(x, Wq, bq, Wk, bk, Wv, bv, Wo, bo):
    scale = 1.0 / np.sqrt(np.float32(HD))
    wq_r = _repack_w((Wq * scale).astype(np.float32))
    wk_r = _repack_w(Wk.astype(np.float32))
    wv_r = _repack_w(Wv.astype(np.float32))
    bq_s = (bq * scale).astype(np.float32)
    in_maps = []
    for c in range(8):
        i, b = c % 2, c // 2
        perm = _perm_blocks(i)
        xbT = x[b].T  # (D, L) view
        xT = np.concatenate(
            [xbT[:, p * BLK:(p + 1) * BLK] for p in perm], axis=1
        )
        # (D, L) -> [128, NRB, ND, 512]: xr[p, rb, dt, c] = xT[dt*128+p,
        # rb*512+c]
        xr = np.ascontiguousarray(
            xT.reshape(ND, 128, NRB, 512).transpose(1, 2, 0, 3)
        ).astype(ml_dtypes.bfloat16)
        biases = np.zeros((128, 8), np.float32)
        biases[:, 0] = bq_s
        biases[:, 1] = bk.astype(np.float32)
        biases[:, 2] = bv.astype(np.float32)
        biases[:, 3] = NEG if i == 0 else 0.0   # phase A, slot kblk=2
        biases[:, 4] = 0.0 if i == 0 else NEG   # phase B, slot kblk=3
        in_maps.append({
            "xr": xr,
            "wq": wq_r,
            "wk": wk_r,
            "wv": wv_r,
            "wo": Wo.astype(ml_dtypes.bfloat16),
            "biases": biases,
        })
    return in_maps


def assemble_output(results, bo):
    out = np.empty((B, L, D), np.float32)
    for c in range(8):
        i, b = c % 2, c // 2
        perm = _perm_blocks(i)
        arr = np.asarray(results[c]["outT"], dtype=np.float32)
        # [128, 4, ND, 512] -> (D, LQ)
        outT = arr.transpose(2, 0, 1, 3).reshape(D, LQ)
        outT /= np.asarray(results[c]["rowsums"], dtype=np.float32)
        qA, qB = perm[0], perm[1]
        out[b, qA * BLK:(qA + 1) * BLK, :] = outT[:, 0:BLK].T
        out[b, qB * BLK:(qB + 1) * BLK, :] = outT[:, BLK:2 * BLK].T
    out += bo.astype(np.float32)
    return out


def kernel(x, Wq, bq, Wk, bk, Wv, bv, Wo, bo):
    from concourse.bass_utils import run_bass_kernel_spmd

    nc = _get_program()
    in_maps = make_in_maps(
        np.asarray(x), np.asarray(Wq), np.asarray(bq), np.asarray(Wk),
        np.asarray(bk), np.asarray(Wv), np.asarray(bv), np.asarray(Wo),
        np.asarray(bo),
    )
    res = run_bass_kernel_spmd(nc, in_maps, core_ids=list(range(8)))
    return assemble_output(res.results, np.asarray(bo))


# revision 33
# speedup vs baseline: 1.1262x; 1.0100x over previous
"""Causal attention (B=4, L=4096, D=2048, HD=128) on 8 TRN2 NeuronCores.

Sharding: 8 cores = 4 batches x 2 fold-halves. Core c handles batch b=c//2
and query blocks {i, 3-i} (1024 rows each) where i=c%2 — the "fold" split
balances causal attention work exactly across the two cores of a batch.
Each core recomputes K/V for all 4096 keys of its batch (no collectives).

The on-device program is identical on all cores (SPMD); per-core behavior
comes only from the data: a block-permuted repacked input xr and two
slot-bias vectors that enable/disable the two fold-dependent key blocks
(bias 0 keeps scores, bias -50 drives exp() to ~1e-22, i.e. masks).

v2 changes vs v1:
  - host repacks x and the weights so every DMA has >=4KB contiguous
    per-partition lines (one DMA per 512-col xt tile instead of 4, one
    per weight); output is written bf16 in [128, qb, dt, 512] layout,
    two DMAs per 512-query block instead of 16.
  - score tiles are computed in PAIRS ([128,1024] PSUM spanning 2 banks)
    so one ACT exp call covers 2 k-tiles, amortizing the 352-cycle ACT
    fixed overhead (1147ns/pair vs 2x720ns).
  - the est tree-adds for row-sums moved from gpsimd to DVE on pairs.
  - attention units take a "filler" generator (projection or out-proj
    chunks) and interleave one chunk per score pair so PE never waits
    on the ACT exp pipeline.

Layouts (partition dim first):
  xr      [128, 8, 16, 512]  x[b].T block-permuted: [p, rb, dt, col]
  Qt, Kt  [HD=128, Lq/Lk]    projections, head dim on partitions
  v_s     [k, HD] slabs      natural V per 128-key tile (PE transpose)
  scores  [k=128, 1024]      two k-tiles per PSUM pair tile; exp on ACT
  outT    [128, 4, 16, 512]  bf16 [p, qb, dt, col]; host divides by
                             row-sums, transposes back, adds bo
"""

import numpy as np
import ml_dtypes

B, L, D, HD = 4, 4096, 2048, 128
BLK = 1024            # fold block (4 per batch)
LQ = 2 * BLK          # queries per core
LK = L                # keys per core
ND = D // 128         # 16 d-tiles
NRB = LK // 512       # 8 column blocks for projections
NEG = -50.0           # slot-disable bias (exp(x-50) ~ 0)
MASKVAL = -30000.0    # intra-tile causal mask additive value

_cached = {}


def _build_program():
    import concourse.bass as bass
    import concourse.tile as tile
    from concourse import bacc, mybir
    from concourse.masks import make_identity

    f32 = mybir.dt.float32
    bf16 = mybir.dt.bfloat16
    nc = bacc.Bacc("TRN2", target_bir_lowering=False, debug=False)

    xr_d = nc.dram_tensor("xr", (128, NRB, ND, 512), bf16,
                          kind="ExternalInput")
    wq_d = nc.dram_tensor("wq", (128, ND, 128), bf16, kind="ExternalInput")
    wk_d = nc.dram_tensor("wk", (128, ND, 128), bf16, kind="ExternalInput")
    wv_d = nc.dram_tensor("wv", (128, ND, 128), bf16, kind="ExternalInput")
    wo_d = nc.dram_tensor("wo", (HD, D), bf16, kind="ExternalInput")
    bias_d = nc.dram_tensor("biases", (128, 8), f32, kind="ExternalInput")
    out_d = nc.dram_tensor("outT", (128, 4, ND, 512), bf16,
                           kind="ExternalOutput")
    rs_d = nc.dram_tensor("rowsums", (1, LQ), f32, kind="ExternalOutput")

    # phase -> list of (local_kblk, kind); kind in {"diag", "full", "bA", "bB"}
    SLOTS = {
        0: [(0, "diag"), (2, "bA")],
        1: [(0, "full"), (1, "diag"), (2, "full"), (3, "bB")],
    }

    with tile.TileContext(nc) as tc:
        with (
            tc.tile_pool(name="const", bufs=1) as cpool,
            tc.tile_pool(name="xt", bufs=3) as xtpool,
            tc.tile_pool(name="vt", bufs=3) as vtpool,
            tc.tile_pool(name="expst", bufs=6) as epool,
            tc.tile_pool(name="outsb", bufs=2) as outpool,
            tc.tile_pool(name="psum", bufs=1, space="PSUM") as psum,
        ):
            # ---- persistent SBUF tensors ----
            wq_s = cpool.tile([128, ND, 128], bf16, tag="wq")
            wk_s = cpool.tile([128, ND, 128], bf16, tag="wk")
            wv_s = cpool.tile([128, ND, 128], bf16, tag="wv")
            wo_s = cpool.tile([128, D], bf16, tag="wo")
            bias_s = cpool.tile([128, 8], f32, tag="biases")
            kt_s = cpool.tile([128, LK], bf16, tag="kt")
            qt_s = cpool.tile([128, LQ], bf16, tag="qt")
            v_s = cpool.tile([128, LK], bf16, tag="v")
            ones_s = cpool.tile([128, 1], bf16, tag="ones")
            rs_s = cpool.tile([1, LQ], f32, tag="rs")
            masks_s = cpool.tile([128, 4 * 512], bf16, tag="masks")
            ot_s = cpool.tile([128, LQ], bf16, tag="ot")
            identb_s = cpool.tile([128, 128], bf16, tag="identb")

            # first xt block + wk first so PE can start ASAP; xt1 ahead of
            # wv/wq so rb1's K can follow rb0 without a DMA underrun; wo is
            # deferred (not needed until the first out-projection).
            xts = {}
            nc.sync.dma_start(wk_s[:], wk_d.ap())
            xts[0] = xtpool.tile([128, ND, 512], bf16, tag="xt", name="xt")
            for ch in range(4):
                nc.sync.dma_start(
                    xts[0][:, ch * 4:(ch + 1) * 4, :],
                    xr_d.ap()[:, 0, ch * 4:(ch + 1) * 4, :],
                )
            nc.sync.dma_start(wv_s[:], wv_d.ap())
            nc.sync.dma_start(wq_s[:], wq_d.ap())
            nc.sync.dma_start(bias_s[:], bias_d.ap())
            xts[1] = xtpool.tile([128, ND, 512], bf16, tag="xt", name="xt")
            nc.sync.dma_start(xts[1][:], xr_d.ap()[:, 1])

            make_identity(nc, identb_s[:])
            nc.gpsimd.memset(ones_s[:], 1.0)
            # preload the ACT exp table during the DMA-bound head (the first
            # real exp would otherwise eat a ~1.3us ACT_TABLE_LOAD stall
            # mid-attention)
            warm = epool.tile([128, 1], bf16, tag="warm", name="warm")
            nc.scalar.activation(
                warm[:], ones_s[:], mybir.ActivationFunctionType.Exp,
                bias=0.0,
            )
            # 4 causal mask tiles for relative offsets delta = 0,128,256,384:
            # keep 0 where q_free >= k_part + delta, else MASKVAL
            nc.gpsimd.memset(masks_s[:], 0.0)
            for m in range(4):
                nc.gpsimd.affine_select(
                    out=masks_s[:, m * 512:(m + 1) * 512],
                    in_=masks_s[:, m * 512:(m + 1) * 512],
                    compare_op=mybir.AluOpType.is_ge,
                    fill=MASKVAL,
                    base=-(m * 128),
                    channel_multiplier=-1,
                    pattern=[[1, 512]],
                )

            bq_ap = bias_s[:, 0:1]
            bk_ap = bias_s[:, 1:2]
            bv_ap = bias_s[:, 2:3]
            slot_bias = {"bA": bias_s[:, 3:4], "bB": bias_s[:, 4:5]}

            def prefetch(rb):
                xts[rb] = xtpool.tile([128, ND, 512], bf16, tag="xt",
                                      name="xt")
                nc.sync.dma_start(xts[rb][:], xr_d.ap()[:, rb])

            def emit_rb_gen(rb, prefetch_rb=None):
                """Projections for one 512-wide column block of xr.
                Yields between ~1us chunks so it can fill attention gaps."""
                xt = xts.pop(rb)
                if prefetch_rb is not None:
                    prefetch(prefetch_rb)
                cs = slice(rb * 512, (rb + 1) * 512)

                pk = psum.tile([128, 512], f32, tag="acc512", bufs=2,
                               name="pk")
                for dt in range(ND):
                    nc.tensor.matmul(
                        pk[:], wk_s[:, dt, :], xt[:, dt, :],
                        start=(dt == 0), stop=(dt == ND - 1),
                    )
                    if dt % 4 == 3:
                        yield
                nc.vector.tensor_scalar_add(kt_s[:, cs], pk[:], bk_ap)

                pv = psum.tile([128, 512], f32, tag="acc512", bufs=2,
                               name="pv")
                for dt in range(ND):
                    nc.tensor.matmul(
                        pv[:], wv_s[:, dt, :], xt[:, dt, :],
                        start=(dt == 0), stop=(dt == ND - 1),
                    )
                    if dt % 4 == 3:
                        yield
                vt_tmp = vtpool.tile([128, 512], bf16, tag="vt_tmp")
                nc.vector.tensor_scalar_add(vt_tmp[:], pv[:], bv_ap)
                for s in range(4):
                    ktile = rb * 4 + s
                    vp = psum.tile([128, 128], bf16, tag="acc512", bufs=2,
                                   name="vp")
                    nc.tensor.transpose(
                        vp[:], vt_tmp[:, s * 128:(s + 1) * 128], identb_s[:]
                    )
                    nc.vector.tensor_copy(
                        v_s[:, ktile * 128:(ktile + 1) * 128], vp[:]
                    )
                yield

                if rb < LQ // 512:
                    pq = psum.tile([128, 512], f32, tag="acc512", bufs=2,
                                   name="pq")
                    for dt in range(ND):
                        nc.tensor.matmul(
                            pq[:], wq_s[:, dt, :], xt[:, dt, :],
                            start=(dt == 0), stop=(dt == ND - 1),
                        )
                        if dt % 4 == 3:
                            yield
                    nc.vector.tensor_scalar_add(qt_s[:, cs], pq[:], bq_ap)

            def emit_rb(rb, prefetch_rb=None):
                for _ in emit_rb_gen(rb, prefetch_rb):
                    pass

            def build_pairs(phase, u):
                """Pairs of k-tiles sharing one exp: (kt_a, kt_b, midx_a,
                bkey). Masked halves use masks_s tile midx_a + h. Diagonal
                pairs go LAST so the first AV matmul (start=True) always
                covers the full query range."""
                pairs = []
                diag_pairs = []
                for kblk, kind in SLOTS[phase]:
                    tiles = []
                    for t in range(8):
                        if kind == "diag":
                            drel = t * 128 - u * 512
                            if drel >= 512:
                                continue
                            midx = drel // 128 if drel >= 0 else None
                            tiles.append((kblk * 8 + t, midx))
                        else:
                            tiles.append((kblk * 8 + t, None))
                    bkey = kind if kind in slot_bias else None
                    dst = diag_pairs if kind == "diag" else pairs
                    # tiles with masks come in runs of consecutive midx
                    i = 0
                    while i < len(tiles):
                        (ta, ma), (tb, mb) = tiles[i], tiles[i + 1]
                        assert (ma is None) == (mb is None)
                        dst.append((ta, tb, ma, bkey))
                        i += 2
                return pairs + diag_pairs

            def emit_attn_u(phase, u, filler=None, nfill=1):
                q0 = phase * BLK + u * 512
                pairs = build_pairs(phase, u)
                n = len(pairs)
                ngroups = n // 2
                ot_acc = psum.tile([128, 512], f32, tag="otacc", bufs=1,
                                   name="ot_acc")
                rs_acc = psum.tile([1, 512], f32, tag="rs", bufs=1,
                                   name="rs_acc")
                ests = [None] * n

                def emit_pair(pi):
                    ta, tb, ma, bkey = pairs[pi]
                    stp = psum.tile([128, 1024], f32, tag="stp", bufs=2,
                                    name="stp")
                    # causal masks are pre-loaded into PSUM via an identity
                    # matmul and the score matmul accumulates on top — this
                    # keeps the mask off the DVE (a [128,1024] PSUM
                    # tensor_add costs ~1.2us and serializes score->exp).
                    # The score matmul then only covers the live query range
                    # (the mask-init already filled the dead zone with
                    # MASKVAL, so exp drives it to 0).
                    for h, kt in ((0, ta), (1, tb)):
                        if ma is not None:
                            m = ma + h
                            nc.tensor.matmul(
                                stp[:, h * 512:(h + 1) * 512], identb_s[:],
                                masks_s[:, m * 512:(m + 1) * 512],
                                start=True, stop=False,
                                skip_group_check=True,
                            )
                            off = m * 128
                            nc.tensor.matmul(
                                stp[:, h * 512 + off:(h + 1) * 512],
                                kt_s[:, kt * 128:(kt + 1) * 128],
                                qt_s[:, q0 + off:q0 + 512],
                                start=False, stop=True,
                                skip_group_check=True,
                            )
                        else:
                            nc.tensor.matmul(
                                stp[:, h * 512:(h + 1) * 512],
                                kt_s[:, kt * 128:(kt + 1) * 128],
                                qt_s[:, q0:q0 + 512],
                                start=True, stop=True,
                            )
                    est = epool.tile([128, 1024], bf16, tag="est")
                    nc.scalar.activation(
                        est[:], stp[:],
                        mybir.ActivationFunctionType.Exp,
                        bias=slot_bias[bkey] if bkey else 0.0,
                    )
                    ests[pi] = est

                emit_pair(0)
                if n > 1:
                    emit_pair(1)
                for pi in range(n):
                    ta, tb, ma, bkey = pairs[pi]
                    if pi + 2 < n:
                        emit_pair(pi + 2)
                    if filler is not None:
                        for _ in range(nfill):
                            next(filler, None)
                    est = ests[pi]
                    # masked (diagonal) pairs only contribute on their live
                    # query range; they are ordered last so never carry
                    # start=True (the dead columns were already written by
                    # full pairs)
                    for h, kt in ((0, ta), (1, tb)):
                        off = 0 if ma is None else (ma + h) * 128
                        nc.tensor.matmul(
                            ot_acc[:, off:512],
                            v_s[:, kt * 128:(kt + 1) * 128],
                            est[:, h * 512 + off:(h + 1) * 512],
                            start=(pi == 0 and h == 0),
                            stop=(pi == n - 1 and h == 1),
                            skip_group_check=True,
                        )
                    if pi % 2 == 1:
                        g = pi // 2
                        esum = epool.tile([128, 1024], bf16, tag="esum",
                                          name="esum")
                        nc.vector.tensor_add(
                            esum[:], ests[pi - 1][:], est[:]
                        )
                        fold = epool.tile([128, 512], bf16, tag="fold",
                                          name="fold")
                        nc.vector.tensor_add(
                            fold[:], esum[:, 0:512], esum[:, 512:1024]
                        )
                        nc.tensor.matmul(
                            rs_acc[:], ones_s[:], fold[:],
                            start=(g == 0), stop=(g == ngroups - 1),
                        )

                qb = phase * 2 + u
                if phase == 1 and u == 1:
                    # last unit: ACT still has queued exps, a split copy
                    # would finish LATER than a single DVE copy
                    nc.vector.tensor_copy(
                        ot_s[:, qb * 512:(qb + 1) * 512], ot_acc[:]
                    )
                else:
                    # split the u-end ot drain across DVE and ACT so the
                    # next consumer (out-proj matmul) unblocks ~2x sooner
                    nc.vector.tensor_copy(
                        ot_s[:, qb * 512:qb * 512 + 256], ot_acc[:, 0:256]
                    )
                    nc.scalar.activation(
                        ot_s[:, qb * 512 + 256:(qb + 1) * 512],
                        ot_acc[:, 256:512],
                        mybir.ActivationFunctionType.Copy,
                    )
                nc.vector.tensor_copy(
                    rs_s[:, qb * 512:(qb + 1) * 512], rs_acc[:]
                )
                if filler is not None:
                    for _ in filler:  # drain unconsumed filler chunks
                        pass

            def outproj_gen(qb, on_act=False, nstores=2, use_stp=False):
                """Out-projection for one 512-query block into a bf16 slab.
                Stores go on the gpsimd SWDGE queue so they never wait
                behind input-prefetch WAR stalls on the sync queue. Yields
                per dt chunk. on_act alternates copies onto ACT (only for
                regions where ACT is not running exp). use_stp borrows the
                stp PSUM banks for a 4-deep drain pipeline — only safe
                after all attention units are done."""
                slab = outpool.tile([128, ND, 512], bf16, tag="oslab",
                                    name="oslab")
                per = ND // nstores
                pop = None
                for dt in range(ND):
                    if use_stp and dt % 4 < 2:
                        # alternate between stp pair-tiles and acc512 so
                        # the drain pipeline is 6 deep after attention
                        if dt % 4 == 0:
                            pop = psum.tile([128, 1024], f32, tag="stp",
                                            bufs=2, name="po")
                        po = pop[:, (dt % 4) * 512:(dt % 4) * 512 + 512]
                    else:
                        po = psum.tile([128, 512], f32, tag="acc512",
                                       bufs=2, name="po")[:]
                    nc.tensor.matmul(
                        po,
                        wo_s[:, dt * 128:(dt + 1) * 128],
                        ot_s[:, qb * 512:(qb + 1) * 512],
                        start=True, stop=True,
                    )
                    if on_act and dt % 2 == 1:
                        nc.scalar.activation(
                            slab[:, dt, :], po,
                            mybir.ActivationFunctionType.Copy,
                        )
                    else:
                        nc.vector.tensor_copy(slab[:, dt, :], po)
                    if dt % per == per - 1:
                        s = dt + 1 - per
                        nc.sync.dma_start(
                            out_d.ap()[:, qb, s:dt + 1], slab[:, s:dt + 1]
                        )
                    yield

            def emit_outproj(qb, on_act=False, nstores=2, use_stp=False):
                for _ in outproj_gen(qb, on_act, nstores, use_stp):
                    pass

            def chain(*gens):
                for g in gens:
                    for x in g:
                        yield x

            def interleave(*gens):
                live = list(gens)
                while live:
                    for g in list(live):
                        try:
                            next(g)
                        except StopIteration:
                            live.remove(g)

            # ---- interleaved schedule ----
            # phase 0 needs local kblks 0 (rbs 0,1) and 2 (rbs 4,5) plus
            # Qt[0:1024) (rbs 0,1); phase 1 needs everything.
            emit_rb(0, prefetch_rb=4)
            nc.sync.dma_start(wo_s[:], wo_d.ap())
            emit_rb(1, prefetch_rb=5)
            emit_rb(4, prefetch_rb=2)
            emit_rb(5, prefetch_rb=3)
            emit_attn_u(0, 0, filler=emit_rb_gen(2, prefetch_rb=6))
            emit_attn_u(0, 1, filler=emit_rb_gen(3, prefetch_rb=7))
            # out-proj of qb0 rides along rb6/rb7: its matmuls interleave
            # with projection matmuls on PE while its PSUM drains use the
            # exp-free DVE/ACT window; attn(1,*) then only carries one
            # out-proj each at 1 chunk/pair so DVE doesn't saturate
            interleave(chain(emit_rb_gen(6), emit_rb_gen(7)),
                       outproj_gen(0, on_act=True))
            emit_attn_u(1, 0, filler=outproj_gen(1), nfill=1)
            emit_attn_u(1, 1, filler=outproj_gen(2), nfill=1)
            emit_outproj(3, on_act=True, nstores=4, use_stp=True)
            nc.sync.dma_start(rs_d.ap(), rs_s[:])

    nc.compile()
    return nc


def _get_program():
    if "nc" not in _cached:
        _cached["nc"] = _build_program()
    return _cached["nc"]


def _perm_blocks(i):
    # local order [qA, qB, o1, o2]
    return [0, 3, 1, 2] if i == 0 else [1, 2, 0, 3]


def _repack_w(w):
    # (D, HD) -> [128, ND, 128] with per-partition contiguous lines
    return np.ascontiguousarray(
        w.reshape(ND, 128, HD).transpose(1, 0, 2)
    ).astype(ml_dtypes.bfloat16)


def make_in_maps(x, Wq, bq, Wk, bk, Wv, bv, Wo, bo):
    scale = 1.0 / np.sqrt(np.float32(HD))
    wq_r = _repack_w((Wq * scale).astype(np.float32))
    wk_r = _repack_w(Wk.astype(np.float32))
    wv_r = _repack_w(Wv.astype(np.float32))
    bq_s = (bq * scale).astype(np.float32)
    in_maps = []
    for c in range(8):
        i, b = c % 2, c // 2
        perm = _perm_blocks(i)
        xbT = x[b].T  # (D, L) view
        xT = np.concatenate(
            [xbT[:, p * BLK:(p + 1) * BLK] for p in perm], axis=1
        )
        # (D, L) -> [128, NRB, ND, 512]: xr[p, rb, dt, c] = xT[dt*128+p,
        # rb*512+c]
        xr = np.ascontiguousarray(
            xT.reshape(ND, 128, NRB, 512).transpose(1, 2, 0, 3)
        ).astype(ml_dtypes.bfloat16)
        biases = np.zeros((128, 8), np.float32)
        biases[:, 0] = bq_s
        biases[:, 1] = bk.astype(np.float32)
        biases[:, 2] = bv.astype(np.float32)
        biases[:, 3] = NEG if i == 0 else 0.0   # phase A, slot kblk=2
        biases[:, 4] = 0.0 if i == 0 else NEG   # phase B, slot kblk=3
        in_maps.append({
            "xr": xr,
            "wq": wq_r,
            "wk": wk_r,
            "wv": wv_r,
            "wo": Wo.astype(ml_dtypes.bfloat16),
            "biases": biases,
        })
    return in_maps


def assemble_output(results, bo):
    out = np.empty((B, L, D), np.float32)
    for c in range(8):
        i, b = c % 2, c // 2
        perm = _perm_blocks(i)
        arr = np.asarray(results[c]["outT"], dtype=np.float32)
        # [128, 4, ND, 512] -> (D, LQ)
        outT = arr.transpose(2, 0, 1, 3).reshape(D, LQ)
        outT /= np.asarray(results[c]["rowsums"], dtype=np.float32)
        qA, qB = perm[0], perm[1]
        out[b, qA * BLK:(qA + 1) * BLK, :] = outT[:, 0:BLK].T
        out[b, qB * BLK:(qB + 1) * BLK, :] = outT[:, BLK:2 * BLK].T
    out += bo.astype(np.float32)
    return out


def kernel(x, Wq, bq, Wk, bk, Wv, bv, Wo, bo):
    from concourse.bass_utils import run_bass_kernel_spmd

    nc = _get_program()
    in_maps = make_in_maps(
        np.asarray(x), np.asarray(Wq), np.asarray(bq), np.asarray(Wk),
        np.asarray(bk), np.asarray(Wv), np.asarray(bv), np.asarray(Wo),
        np.asarray(bo),
    )
    res = run_bass_kernel_spmd(nc, in_maps, core_ids=list(range(8)))
    return assemble_output(res.results, np.asarray(bo))


# revision 37
# speedup vs baseline: 1.1459x; 1.0174x over previous
"""Causal attention (B=4, L=4096, D=2048, HD=128) on 8 TRN2 NeuronCores.

Sharding: 8 cores = 4 batches x 2 fold-halves. Core c handles batch b=c//2
and query blocks {i, 3-i} (1024 rows each) where i=c%2 — the "fold" split
balances causal attention work exactly across the two cores of a batch.
Each core recomputes K/V for all 4096 keys of its batch (no collectives).

The on-device program is identical on all cores (SPMD); per-core behavior
comes only from the data: a block-permuted repacked input xr and two
slot-bias vectors that enable/disable the two fold-dependent key blocks
(bias 0 keeps scores, bias -50 drives exp() to ~1e-22, i.e. masks).

v2 changes vs v1:
  - host repacks x and the weights so every DMA has >=4KB contiguous
    per-partition lines (one DMA per 512-col xt tile instead of 4, one
    per weight); output is written bf16 in [128, qb, dt, 512] layout,
    two DMAs per 512-query block instead of 16.
  - score tiles are computed in PAIRS ([128,1024] PSUM spanning 2 banks)
    so one ACT exp call covers 2 k-tiles, amortizing the 352-cycle ACT
    fixed overhead (1147ns/pair vs 2x720ns).
  - the est tree-adds for row-sums moved from gpsimd to DVE on pairs.
  - attention units take a "filler" generator (projection or out-proj
    chunks) and interleave one chunk per score pair so PE never waits
    on the ACT exp pipeline.

Layouts (partition dim first):
  xr      [128, 8, 16, 512]  x[b].T block-permuted: [p, rb, dt, col]
  Qt, Kt  [HD=128, Lq/Lk]    projections, head dim on partitions
  v_s     [k, HD] slabs      natural V per 128-key tile (PE transpose)
  scores  [k=128, 1024]      two k-tiles per PSUM pair tile; exp on ACT
  outT    [128, 4, 16, 512]  bf16 [p, qb, dt, col]; host divides by
                             row-sums, transposes back, adds bo
"""

import numpy as np
import ml_dtypes

B, L, D, HD = 4, 4096, 2048, 128
BLK = 1024            # fold block (4 per batch)
LQ = 2 * BLK          # queries per core
LK = L                # keys per core
ND = D // 128         # 16 d-tiles
NRB = LK // 512       # 8 column blocks for projections
NEG = -50.0           # slot-disable bias (exp(x-50) ~ 0)
MASKVAL = -30000.0    # intra-tile causal mask additive value

_cached = {}


def _build_program():
    import concourse.bass as bass
    import concourse.tile as tile
    from concourse import bacc, mybir
    from concourse.masks import make_identity

    f32 = mybir.dt.float32
    bf16 = mybir.dt.bfloat16
    nc = bacc.Bacc("TRN2", target_bir_lowering=False, debug=False)

    xr_d = nc.dram_tensor("xr", (128, NRB, ND, 512), bf16,
                          kind="ExternalInput")
    wq_d = nc.dram_tensor("wq", (128, ND, 128), bf16, kind="ExternalInput")
    wk_d = nc.dram_tensor("wk", (128, ND, 128), bf16, kind="ExternalInput")
    wv_d = nc.dram_tensor("wv", (128, ND, 128), bf16, kind="ExternalInput")
    wo_d = nc.dram_tensor("wo", (HD, D), bf16, kind="ExternalInput")
    bias_d = nc.dram_tensor("biases", (128, 8), f32, kind="ExternalInput")
    out_d = nc.dram_tensor("outT", (128, 4, ND, 512), bf16,
                           kind="ExternalOutput")
    rs_d = nc.dram_tensor("rowsums", (1, LQ), f32, kind="ExternalOutput")

    # phase -> list of (local_kblk, kind); kind in {"diag", "full", "bA", "bB"}
    SLOTS = {
        0: [(0, "diag"), (2, "bA")],
        1: [(0, "full"), (1, "diag"), (2, "full"), (3, "bB")],
    }

    with tile.TileContext(nc) as tc:
        with (
            tc.tile_pool(name="const", bufs=1) as cpool,
            tc.tile_pool(name="xt", bufs=3) as xtpool,
            tc.tile_pool(name="vt", bufs=3) as vtpool,
            tc.tile_pool(name="expst", bufs=6) as epool,
            tc.tile_pool(name="outsb", bufs=2) as outpool,
            tc.tile_pool(name="psum", bufs=1, space="PSUM") as psum,
        ):
            # ---- persistent SBUF tensors ----
            wq_s = cpool.tile([128, ND, 128], bf16, tag="wq")
            wk_s = cpool.tile([128, ND, 128], bf16, tag="wk")
            wv_s = cpool.tile([128, ND, 128], bf16, tag="wv")
            wo_s = cpool.tile([128, D], bf16, tag="wo")
            bias_s = cpool.tile([128, 8], f32, tag="biases")
            kt_s = cpool.tile([128, LK], bf16, tag="kt")
            qt_s = cpool.tile([128, LQ], bf16, tag="qt")
            v_s = cpool.tile([128, LK], bf16, tag="v")
            ones_s = cpool.tile([128, 1], bf16, tag="ones")
            rs_s = cpool.tile([1, LQ], f32, tag="rs")
            masks_s = cpool.tile([128, 4 * 512], bf16, tag="masks")
            ot_s = cpool.tile([128, LQ], bf16, tag="ot")
            identb_s = cpool.tile([128, 128], bf16, tag="identb")

            # first xt block + wk first so PE can start ASAP; xt1 ahead of
            # wv/wq so rb1's K can follow rb0 without a DMA underrun; wo is
            # deferred (not needed until the first out-projection).
            xts = {}
            nc.sync.dma_start(wk_s[:], wk_d.ap())
            xts[0] = xtpool.tile([128, ND, 512], bf16, tag="xt", name="xt")
            for ch in range(4):
                nc.sync.dma_start(
                    xts[0][:, ch * 4:(ch + 1) * 4, :],
                    xr_d.ap()[:, 0, ch * 4:(ch + 1) * 4, :],
                )
            nc.sync.dma_start(wv_s[:], wv_d.ap())
            nc.sync.dma_start(wq_s[:], wq_d.ap())
            nc.sync.dma_start(bias_s[:], bias_d.ap())
            xts[1] = xtpool.tile([128, ND, 512], bf16, tag="xt", name="xt")
            nc.sync.dma_start(xts[1][:, 0:8, :], xr_d.ap()[:, 1, 0:8, :])
            nc.sync.dma_start(xts[1][:, 8:16, :], xr_d.ap()[:, 1, 8:16, :])

            make_identity(nc, identb_s[:])
            nc.gpsimd.memset(ones_s[:], 1.0)
            # ~3.4us of dummy transposes while the first xt DMAs land: the
            # PE_HAM clock gate needs one busy 4096-cycle window before it
            # releases 2.4GHz, so warm it on garbage instead of on the
            # first ~30 real matmuls
            for w in range(32):
                wp = psum.tile([128, 128], bf16, tag="acc512", bufs=2,
                               name="warmmm")
                nc.tensor.transpose(wp[:], identb_s[:], identb_s[:])
            # preload the ACT exp table during the DMA-bound head (the first
            # real exp would otherwise eat a ~1.3us ACT_TABLE_LOAD stall
            # mid-attention)
            warm = epool.tile([128, 1], bf16, tag="warm", name="warm")
            nc.scalar.activation(
                warm[:], ones_s[:], mybir.ActivationFunctionType.Exp,
                bias=0.0,
            )
            # 4 causal mask tiles for relative offsets delta = 0,128,256,384:
            # keep 0 where q_free >= k_part + delta, else MASKVAL
            nc.gpsimd.memset(masks_s[:], 0.0)
            for m in range(4):
                nc.gpsimd.affine_select(
                    out=masks_s[:, m * 512:(m + 1) * 512],
                    in_=masks_s[:, m * 512:(m + 1) * 512],
                    compare_op=mybir.AluOpType.is_ge,
                    fill=MASKVAL,
                    base=-(m * 128),
                    channel_multiplier=-1,
                    pattern=[[1, 512]],
                )

            bq_ap = bias_s[:, 0:1]
            bk_ap = bias_s[:, 1:2]
            bv_ap = bias_s[:, 2:3]
            slot_bias = {"bA": bias_s[:, 3:4], "bB": bias_s[:, 4:5]}

            def prefetch(rb):
                # two half-tile DMAs: the consumer's first 8 d-tile matmuls
                # unblock ~2.8us before the full 2MB tile would land
                xts[rb] = xtpool.tile([128, ND, 512], bf16, tag="xt",
                                      name="xt")
                nc.sync.dma_start(xts[rb][:, 0:8, :],
                                  xr_d.ap()[:, rb, 0:8, :])
                nc.sync.dma_start(xts[rb][:, 8:16, :],
                                  xr_d.ap()[:, rb, 8:16, :])

            def emit_rb_gen(rb, prefetch_rb=None):
                """Projections for one 512-wide column block of xr.
                Yields between ~1us chunks so it can fill attention gaps."""
                xt = xts.pop(rb)
                if prefetch_rb is not None:
                    prefetch(prefetch_rb)
                cs = slice(rb * 512, (rb + 1) * 512)

                pk = psum.tile([128, 512], f32, tag="acc512", bufs=2,
                               name="pk")
                for dt in range(ND):
                    nc.tensor.matmul(
                        pk[:], wk_s[:, dt, :], xt[:, dt, :],
                        start=(dt == 0), stop=(dt == ND - 1),
                    )
                    if dt % 4 == 3:
                        yield
                nc.vector.tensor_scalar_add(kt_s[:, cs], pk[:], bk_ap)

                pv = psum.tile([128, 512], f32, tag="acc512", bufs=2,
                               name="pv")
                for dt in range(ND):
                    nc.tensor.matmul(
                        pv[:], wv_s[:, dt, :], xt[:, dt, :],
                        start=(dt == 0), stop=(dt == ND - 1),
                    )
                    if dt % 4 == 3:
                        yield
                vt_tmp = vtpool.tile([128, 512], bf16, tag="vt_tmp")
                nc.vector.tensor_scalar_add(vt_tmp[:], pv[:], bv_ap)
                for s in range(4):
                    ktile = rb * 4 + s
                    vp = psum.tile([128, 128], bf16, tag="acc512", bufs=2,
                                   name="vp")
                    nc.tensor.transpose(
                        vp[:], vt_tmp[:, s * 128:(s + 1) * 128], identb_s[:]
                    )
                    nc.vector.tensor_copy(
                        v_s[:, ktile * 128:(ktile + 1) * 128], vp[:]
                    )
                yield

                if rb < LQ // 512:
                    pq = psum.tile([128, 512], f32, tag="acc512", bufs=2,
                                   name="pq")
                    for dt in range(ND):
                        nc.tensor.matmul(
                            pq[:], wq_s[:, dt, :], xt[:, dt, :],
                            start=(dt == 0), stop=(dt == ND - 1),
                        )
                        if dt % 4 == 3:
                            yield
                    nc.vector.tensor_scalar_add(qt_s[:, cs], pq[:], bq_ap)

            def emit_rb(rb, prefetch_rb=None):
                for _ in emit_rb_gen(rb, prefetch_rb):
                    pass

            def build_pairs(phase, u):
                """Pairs of k-tiles sharing one exp: (kt_a, kt_b, midx_a,
                bkey). Masked halves use masks_s tile midx_a + h. Diagonal
                pairs go LAST so the first AV matmul (start=True) always
                covers the full query range."""
                pairs = []
                diag_pairs = []
                for kblk, kind in SLOTS[phase]:
                    tiles = []
                    for t in range(8):
                        if kind == "diag":
                            drel = t * 128 - u * 512
                            if drel >= 512:
                                continue
                            midx = drel // 128 if drel >= 0 else None
                            tiles.append((kblk * 8 + t, midx))
                        else:
                            tiles.append((kblk * 8 + t, None))
                    bkey = kind if kind in slot_bias else None
                    dst = diag_pairs if kind == "diag" else pairs
                    # tiles with masks come in runs of consecutive midx
                    i = 0
                    while i < len(tiles):
                        (ta, ma), (tb, mb) = tiles[i], tiles[i + 1]
                        assert (ma is None) == (mb is None)
                        dst.append((ta, tb, ma, bkey))
                        i += 2
                return pairs + diag_pairs

            def emit_attn_u(phase, u, filler=None, nfill=1):
                q0 = phase * BLK + u * 512
                pairs = build_pairs(phase, u)
                n = len(pairs)
                ngroups = n // 2
                ot_acc = psum.tile([128, 512], f32, tag="otacc", bufs=1,
                                   name="ot_acc")
                rs_acc = psum.tile([1, 512], f32, tag="rs", bufs=1,
                                   name="rs_acc")
                ests = [None] * n

                def emit_pair(pi):
                    ta, tb, ma, bkey = pairs[pi]
                    stp = psum.tile([128, 1024], f32, tag="stp", bufs=2,
                                    name="stp")
                    # causal masks are pre-loaded into PSUM via an identity
                    # matmul and the score matmul accumulates on top — this
                    # keeps the mask off the DVE (a [128,1024] PSUM
                    # tensor_add costs ~1.2us and serializes score->exp).
                    # The score matmul then only covers the live query range
                    # (the mask-init already filled the dead zone with
                    # MASKVAL, so exp drives it to 0).
                    for h, kt in ((0, ta), (1, tb)):
                        if ma is not None:
                            m = ma + h
                            nc.tensor.matmul(
                                stp[:, h * 512:(h + 1) * 512], identb_s[:],
                                masks_s[:, m * 512:(m + 1) * 512],
                                start=True, stop=False,
                                skip_group_check=True,
                            )
                            off = m * 128
                            nc.tensor.matmul(
                                stp[:, h * 512 + off:(h + 1) * 512],
                                kt_s[:, kt * 128:(kt + 1) * 128],
                                qt_s[:, q0 + off:q0 + 512],
                                start=False, stop=True,
                                skip_group_check=True,
                            )
                        else:
                            nc.tensor.matmul(
                                stp[:, h * 512:(h + 1) * 512],
                                kt_s[:, kt * 128:(kt + 1) * 128],
                                qt_s[:, q0:q0 + 512],
                                start=True, stop=True,
                            )
                    est = epool.tile([128, 1024], bf16, tag="est")
                    nc.scalar.activation(
                        est[:], stp[:],
                        mybir.ActivationFunctionType.Exp,
                        bias=slot_bias[bkey] if bkey else 0.0,
                    )
                    ests[pi] = est

                emit_pair(0)
                if n > 1:
                    emit_pair(1)
                for pi in range(n):
                    ta, tb, ma, bkey = pairs[pi]
                    if pi + 2 < n:
                        emit_pair(pi + 2)
                    if filler is not None:
                        for _ in range(nfill):
                            next(filler, None)
                    est = ests[pi]
                    # masked (diagonal) pairs only contribute on their live
                    # query range; they are ordered last so never carry
                    # start=True (the dead columns were already written by
                    # full pairs)
                    for h, kt in ((0, ta), (1, tb)):
                        off = 0 if ma is None else (ma + h) * 128
                        nc.tensor.matmul(
                            ot_acc[:, off:512],
                            v_s[:, kt * 128:(kt + 1) * 128],
                            est[:, h * 512 + off:(h + 1) * 512],
                            start=(pi == 0 and h == 0),
                            stop=(pi == n - 1 and h == 1),
                            skip_group_check=True,
                        )
                    if pi % 2 == 1:
                        g = pi // 2
                        esum = epool.tile([128, 1024], bf16, tag="esum",
                                          name="esum")
                        nc.vector.tensor_add(
                            esum[:], ests[pi - 1][:], est[:]
                        )
                        fold = epool.tile([128, 512], bf16, tag="fold",
                                          name="fold")
                        nc.vector.tensor_add(
                            fold[:], esum[:, 0:512], esum[:, 512:1024]
                        )
                        nc.tensor.matmul(
                            rs_acc[:], ones_s[:], fold[:],
                            start=(g == 0), stop=(g == ngroups - 1),
                        )

                qb = phase * 2 + u
                if phase == 1 and u == 1:
                    # last unit: ACT still has queued exps, a split copy
                    # would finish LATER than a single DVE copy
                    nc.vector.tensor_copy(
                        ot_s[:, qb * 512:(qb + 1) * 512], ot_acc[:]
                    )
                else:
                    # split the u-end ot drain across DVE and ACT so the
                    # next consumer (out-proj matmul) unblocks ~2x sooner
                    nc.vector.tensor_copy(
                        ot_s[:, qb * 512:qb * 512 + 256], ot_acc[:, 0:256]
                    )
                    nc.scalar.activation(
                        ot_s[:, qb * 512 + 256:(qb + 1) * 512],
                        ot_acc[:, 256:512],
                        mybir.ActivationFunctionType.Copy,
                    )
                nc.vector.tensor_copy(
                    rs_s[:, qb * 512:(qb + 1) * 512], rs_acc[:]
                )
                if filler is not None:
                    for _ in filler:  # drain unconsumed filler chunks
                        pass

            def outproj_gen(qb, on_act=False, nstores=2, use_stp=False):
                """Out-projection for one 512-query block into a bf16 slab.
                Stores go on the gpsimd SWDGE queue so they never wait
                behind input-prefetch WAR stalls on the sync queue. Yields
                per dt chunk. on_act alternates copies onto ACT (only for
                regions where ACT is not running exp). use_stp borrows the
                stp PSUM banks for a 4-deep drain pipeline — only safe
                after all attention units are done."""
                slab = outpool.tile([128, ND, 512], bf16, tag="oslab",
                                    name="oslab")
                per = ND // nstores
                pop = None
                for dt in range(ND):
                    if use_stp and dt % 4 < 2:
                        # alternate between stp pair-tiles and acc512 so
                        # the drain pipeline is 6 deep after attention
                        if dt % 4 == 0:
                            pop = psum.tile([128, 1024], f32, tag="stp",
                                            bufs=2, name="po")
                        po = pop[:, (dt % 4) * 512:(dt % 4) * 512 + 512]
                    else:
                        po = psum.tile([128, 512], f32, tag="acc512",
                                       bufs=2, name="po")[:]
                    nc.tensor.matmul(
                        po,
                        wo_s[:, dt * 128:(dt + 1) * 128],
                        ot_s[:, qb * 512:(qb + 1) * 512],
                        start=True, stop=True,
                    )
                    if on_act and dt % 2 == 1:
                        nc.scalar.activation(
                            slab[:, dt, :], po,
                            mybir.ActivationFunctionType.Copy,
                        )
                    else:
                        nc.vector.tensor_copy(slab[:, dt, :], po)
                    if dt % per == per - 1:
                        s = dt + 1 - per
                        nc.sync.dma_start(
                            out_d.ap()[:, qb, s:dt + 1], slab[:, s:dt + 1]
                        )
                    yield

            def emit_outproj(qb, on_act=False, nstores=2, use_stp=False):
                for _ in outproj_gen(qb, on_act, nstores, use_stp):
                    pass

            def chain(*gens):
                for g in gens:
                    for x in g:
                        yield x

            def interleave(*gens):
                live = list(gens)
                while live:
                    for g in list(live):
                        try:
                            next(g)
                        except StopIteration:
                            live.remove(g)

            # ---- interleaved schedule ----
            # phase 0 needs local kblks 0 (rbs 0,1) and 2 (rbs 4,5) plus
            # Qt[0:1024) (rbs 0,1); phase 1 needs everything.
            emit_rb(0, prefetch_rb=4)
            nc.sync.dma_start(wo_s[:], wo_d.ap())
            emit_rb(1, prefetch_rb=5)
            emit_rb(4, prefetch_rb=2)
            emit_rb(5, prefetch_rb=3)
            emit_attn_u(0, 0, filler=emit_rb_gen(2, prefetch_rb=6))
            emit_attn_u(0, 1, filler=emit_rb_gen(3, prefetch_rb=7))
            # out-proj of qb0 rides along rb6/rb7: its matmuls interleave
            # with projection matmuls on PE while its PSUM drains use the
            # exp-free DVE/ACT window; attn(1,*) then only carries one
            # out-proj each at 1 chunk/pair so DVE doesn't saturate
            interleave(chain(emit_rb_gen(6), emit_rb_gen(7)),
                       outproj_gen(0, on_act=True))
            emit_attn_u(1, 0, filler=outproj_gen(1), nfill=1)
            emit_attn_u(1, 1, filler=outproj_gen(2), nfill=1)
            nc.sync.dma_start(rs_d.ap(), rs_s[:])
            emit_outproj(3, on_act=True, nstores=4, use_stp=True)

    nc.compile()
    return nc


def _get_program():
    if "nc" not in _cached:
        _cached["nc"] = _build_program()
    return _cached["nc"]


def _perm_blocks(i):
    # local order [qA, qB, o1, o2]
    return [0, 3, 1, 2] if i == 0 else [1, 2, 0, 3]


def _repack_w(w):
    # (D, HD) -> [128, ND, 128] with per-partition contiguous lines
    return np.ascontiguousarray(
        w.reshape(ND, 128, HD).transpose(1, 0, 2)
    ).astype(ml_dtypes.bfloat16)


def make_in_maps(x, Wq, bq, Wk, bk, Wv, bv, Wo, bo):
    scale = 1.0 / np.sqrt(np.float32(HD))
    wq_r = _repack_w((Wq * scale).astype(np.float32))
    wk_r = _repack_w(Wk.astype(np.float32))
    wv_r = _repack_w(Wv.astype(np.float32))
    bq_s = (bq * scale).astype(np.float32)
    in_maps = []
    for c in range(8):
        i, b = c % 2, c // 2
        perm = _perm_blocks(i)
        xbT = x[b].T  # (D, L) view
        xT = np.concatenate(
            [xbT[:, p * BLK:(p + 1) * BLK] for p in perm], axis=1
        )
        # (D, L) -> [128, NRB, ND, 512]: xr[p, rb, dt, c] = xT[dt*128+p,
        # rb*512+c]
        xr = np.ascontiguousarray(
            xT.reshape(ND, 128, NRB, 512).transpose(1, 2, 0, 3)
        ).astype(ml_dtypes.bfloat16)
        biases = np.zeros((128, 8), np.float32)
        biases[:, 0] = bq_s
        biases[:, 1] = bk.astype(np.float32)
        biases[:, 2] = bv.astype(np.float32)
        biases[:, 3] = NEG if i == 0 else 0.0   # phase A, slot kblk=2
        biases[:, 4] = 0.0 if i == 0 else NEG   # phase B, slot kblk=3
        in_maps.append({
            "xr": xr,
            "wq": wq_r,
            "wk": wk_r,
            "wv": wv_r,
            "wo": Wo.astype(ml_dtypes.bfloat16),
            "biases": biases,
        })
    return in_maps


def assemble_output(results, bo):
    out = np.empty((B, L, D), np.float32)
    for c in range(8):
        i, b = c % 2, c // 2
        perm = _perm_blocks(i)
        arr = np.asarray(results[c]["outT"], dtype=np.float32)
        # [128, 4, ND, 512] -> (D, LQ)
        outT = arr.transpose(2, 0, 1, 3).reshape(D, LQ)
        outT /= np.asarray(results[c]["rowsums"], dtype=np.float32)
        qA, qB = perm[0], perm[1]
        out[b, qA * BLK:(qA + 1) * BLK, :] = outT[:, 0:BLK].T
        out[b, qB * BLK:(qB + 1) * BLK, :] = outT[:, BLK:2 * BLK].T
    out += bo.astype(np.float32)
    return out


def kernel(x, Wq, bq, Wk, bk, Wv, bv, Wo, bo):
    from concourse.bass_utils import run_bass_kernel_spmd

    nc = _get_program()
    in_maps = make_in_maps(
        np.asarray(x), np.asarray(Wq), np.asarray(bq), np.asarray(Wk),
        np.asarray(bk), np.asarray(Wv), np.asarray(bv), np.asarray(Wo),
        np.asarray(bo),
    )
    res = run_bass_kernel_spmd(nc, in_maps, core_ids=list(range(8)))
    return assemble_output(res.results, np.asarray(bo))


# revision 41
# speedup vs baseline: 1.1725x; 1.0233x over previous
"""Causal attention (B=4, L=4096, D=2048, HD=128) on 8 TRN2 NeuronCores.

Sharding: 8 cores = 4 batches x 2 fold-halves. Core c handles batch b=c//2
and query blocks {i, 3-i} (1024 rows each) where i=c%2 — the "fold" split
balances causal attention work exactly across the two cores of a batch.
Each core recomputes K/V for all 4096 keys of its batch (no collectives).

The on-device program is identical on all cores (SPMD); per-core behavior
comes only from the data: a block-permuted repacked input xr and two
slot-bias vectors that enable/disable the two fold-dependent key blocks
(bias 0 keeps scores, bias -50 drives exp() to ~1e-22, i.e. masks).

v2 changes vs v1:
  - host repacks x and the weights so every DMA has >=4KB contiguous
    per-partition lines (one DMA per 512-col xt tile instead of 4, one
    per weight); output is written bf16 in [128, qb, dt, 512] layout,
    two DMAs per 512-query block instead of 16.
  - score tiles are computed in PAIRS ([128,1024] PSUM spanning 2 banks)
    so one ACT exp call covers 2 k-tiles, amortizing the 352-cycle ACT
    fixed overhead (1147ns/pair vs 2x720ns).
  - the est tree-adds for row-sums moved from gpsimd to DVE on pairs.
  - attention units take a "filler" generator (projection or out-proj
    chunks) and interleave one chunk per score pair so PE never waits
    on the ACT exp pipeline.

Layouts (partition dim first):
  xr      [128, 8, 16, 512]  x[b].T block-permuted: [p, rb, dt, col]
  Qt, Kt  [HD=128, Lq/Lk]    projections, head dim on partitions
  v_s     [k, HD] slabs      natural V per 128-key tile (PE transpose)
  scores  [k=128, 1024]      two k-tiles per PSUM pair tile; exp on ACT
  outT    [128, 4, 16, 512]  bf16 [p, qb, dt, col]; host divides by
                             row-sums, transposes back, adds bo
"""

import numpy as np
import ml_dtypes

B, L, D, HD = 4, 4096, 2048, 128
BLK = 1024            # fold block (4 per batch)
LQ = 2 * BLK          # queries per core
LK = L                # keys per core
ND = D // 128         # 16 d-tiles
NRB = LK // 512       # 8 column blocks for projections
NEG = -50.0           # slot-disable bias (exp(x-50) ~ 0)
MASKVAL = -30000.0    # intra-tile causal mask additive value

_cached = {}


def _build_program():
    import concourse.bass as bass
    import concourse.tile as tile
    from concourse import bacc, mybir
    from concourse.masks import make_identity

    f32 = mybir.dt.float32
    bf16 = mybir.dt.bfloat16
    nc = bacc.Bacc("TRN2", target_bir_lowering=False, debug=False)

    xr_d = nc.dram_tensor("xr", (128, NRB, ND, 512), bf16,
                          kind="ExternalInput")
    wq_d = nc.dram_tensor("wq", (128, ND, 128), bf16, kind="ExternalInput")
    wk_d = nc.dram_tensor("wk", (128, ND, 128), bf16, kind="ExternalInput")
    wv_d = nc.dram_tensor("wv", (128, ND, 128), bf16, kind="ExternalInput")
    wo_d = nc.dram_tensor("wo", (HD, D), bf16, kind="ExternalInput")
    bias_d = nc.dram_tensor("biases", (128, 8), f32, kind="ExternalInput")
    out_d = nc.dram_tensor("outT", (128, 4, ND, 512), bf16,
                           kind="ExternalOutput")
    rs_d = nc.dram_tensor("rowsums", (1, LQ), f32, kind="ExternalOutput")

    # phase -> list of (local_kblk, kind); kind in {"diag", "full", "bA", "bB"}
    SLOTS = {
        0: [(0, "diag"), (2, "bA")],
        1: [(0, "full"), (1, "diag"), (2, "full"), (3, "bB")],
    }

    with tile.TileContext(nc) as tc:
        with (
            tc.tile_pool(name="const", bufs=1) as cpool,
            tc.tile_pool(name="xt", bufs=3) as xtpool,
            tc.tile_pool(name="vt", bufs=3) as vtpool,
            tc.tile_pool(name="expst", bufs=6) as epool,
            tc.tile_pool(name="outsb", bufs=2) as outpool,
            tc.tile_pool(name="psum", bufs=1, space="PSUM") as psum,
        ):
            # ---- persistent SBUF tensors ----
            wq_s = cpool.tile([128, ND, 128], bf16, tag="wq")
            wk_s = cpool.tile([128, ND, 128], bf16, tag="wk")
            wv_s = cpool.tile([128, ND, 128], bf16, tag="wv")
            wo_s = cpool.tile([128, D], bf16, tag="wo")
            bias_s = cpool.tile([128, 8], f32, tag="biases")
            kt_s = cpool.tile([128, LK], bf16, tag="kt")
            qt_s = cpool.tile([128, LQ], bf16, tag="qt")
            v_s = cpool.tile([128, LK], bf16, tag="v")
            ones_s = cpool.tile([128, 1], bf16, tag="ones")
            rs_s = cpool.tile([1, LQ], f32, tag="rs")
            masks_s = cpool.tile([128, 4 * 512], bf16, tag="masks")
            ot_s = cpool.tile([128, LQ], bf16, tag="ot")
            identb_s = cpool.tile([128, 128], bf16, tag="identb")

            # first xt block + wk first so PE can start ASAP; xt1 ahead of
            # wv/wq so rb1's K can follow rb0 without a DMA underrun; wo is
            # deferred (not needed until the first out-projection).
            xts = {}
            nc.sync.dma_start(wk_s[:], wk_d.ap())
            xts[0] = xtpool.tile([128, ND, 512], bf16, tag="xt", name="xt")
            for ch in range(4):
                nc.sync.dma_start(
                    xts[0][:, ch * 4:(ch + 1) * 4, :],
                    xr_d.ap()[:, 0, ch * 4:(ch + 1) * 4, :],
                )
            nc.sync.dma_start(wv_s[:], wv_d.ap())
            nc.sync.dma_start(wq_s[:], wq_d.ap())
            nc.sync.dma_start(bias_s[:], bias_d.ap())
            xts[1] = xtpool.tile([128, ND, 512], bf16, tag="xt", name="xt")
            nc.sync.dma_start(xts[1][:, 0:8, :], xr_d.ap()[:, 1, 0:8, :])
            nc.sync.dma_start(xts[1][:, 8:16, :], xr_d.ap()[:, 1, 8:16, :])

            make_identity(nc, identb_s[:])
            nc.gpsimd.memset(ones_s[:], 1.0)
            # ~3.4us of dummy transposes while the first xt DMAs land: the
            # PE_HAM clock gate needs one busy 4096-cycle window before it
            # releases 2.4GHz, so warm it on garbage instead of on the
            # first ~30 real matmuls
            for w in range(32):
                wp = psum.tile([128, 128], bf16, tag="acc512", bufs=2,
                               name="warmmm")
                nc.tensor.transpose(wp[:], identb_s[:], identb_s[:])
            # preload the ACT exp table during the DMA-bound head (the first
            # real exp would otherwise eat a ~1.3us ACT_TABLE_LOAD stall
            # mid-attention)
            warm = epool.tile([128, 1], bf16, tag="warm", name="warm")
            nc.scalar.activation(
                warm[:], ones_s[:], mybir.ActivationFunctionType.Exp,
                bias=0.0,
            )
            # 4 causal mask tiles for relative offsets delta = 0,128,256,384:
            # keep 0 where q_free >= k_part + delta, else MASKVAL
            nc.gpsimd.memset(masks_s[:], 0.0)
            for m in range(4):
                nc.gpsimd.affine_select(
                    out=masks_s[:, m * 512:(m + 1) * 512],
                    in_=masks_s[:, m * 512:(m + 1) * 512],
                    compare_op=mybir.AluOpType.is_ge,
                    fill=MASKVAL,
                    base=-(m * 128),
                    channel_multiplier=-1,
                    pattern=[[1, 512]],
                )

            bq_ap = bias_s[:, 0:1]
            bk_ap = bias_s[:, 1:2]
            bv_ap = bias_s[:, 2:3]
            slot_bias = {"bA": bias_s[:, 3:4], "bB": bias_s[:, 4:5]}

            def prefetch(rb):
                # two half-tile DMAs: the consumer's first 8 d-tile matmuls
                # unblock ~2.8us before the full 2MB tile would land
                xts[rb] = xtpool.tile([128, ND, 512], bf16, tag="xt",
                                      name="xt")
                nc.sync.dma_start(xts[rb][:, 0:8, :],
                                  xr_d.ap()[:, rb, 0:8, :])
                nc.sync.dma_start(xts[rb][:, 8:16, :],
                                  xr_d.ap()[:, rb, 8:16, :])

            def emit_rb_gen(rb, prefetch_rb=None):
                """Projections for one 512-wide column block of xr.
                Yields between ~1us chunks so it can fill attention gaps."""
                xt = xts.pop(rb)
                if prefetch_rb is not None:
                    prefetch(prefetch_rb)
                cs = slice(rb * 512, (rb + 1) * 512)

                pk = psum.tile([128, 512], f32, tag="acc512", bufs=2,
                               name="pk")
                for dt in range(ND):
                    nc.tensor.matmul(
                        pk[:], wk_s[:, dt, :], xt[:, dt, :],
                        start=(dt == 0), stop=(dt == ND - 1),
                    )
                    if dt % 4 == 3:
                        yield
                nc.vector.tensor_scalar_add(kt_s[:, cs], pk[:], bk_ap)

                pv = psum.tile([128, 512], f32, tag="acc512", bufs=2,
                               name="pv")
                for dt in range(ND):
                    nc.tensor.matmul(
                        pv[:], wv_s[:, dt, :], xt[:, dt, :],
                        start=(dt == 0), stop=(dt == ND - 1),
                    )
                    if dt % 4 == 3:
                        yield
                # all 4 transposes land in ONE bf16 PSUM tile (quarters stay
                # within a bank), drained by a single DVE copy; allocation
                # order is chosen so no matmul ever inherits a WAR wait on a
                # still-pending DVE drain
                vt_tmp = vtpool.tile([128, 512], bf16, tag="vt_tmp")

                def transposes(vp4, lo, hi):
                    for s in range(lo, hi):
                        nc.tensor.transpose(
                            vp4[:, s * 128:(s + 1) * 128],
                            vt_tmp[:, s * 128:(s + 1) * 128],
                            identb_s[:],
                        )

                if rb < LQ // 512:
                    # Q's matmuls run while the DVE drains pv into vt_tmp,
                    # so the transposes never stall the PE on the drain
                    nc.vector.tensor_scalar_add(vt_tmp[:], pv[:], bv_ap)
                    pq = psum.tile([128, 512], f32, tag="acc512", bufs=2,
                                   name="pq")
                    for dt in range(ND):
                        nc.tensor.matmul(
                            pq[:], wq_s[:, dt, :], xt[:, dt, :],
                            start=(dt == 0), stop=(dt == ND - 1),
                        )
                        if dt % 4 == 3:
                            yield
                    vp4 = psum.tile([128, 512], bf16, tag="acc512", bufs=2,
                                    name="vp4")
                    transposes(vp4, 0, 4)
                    nc.vector.tensor_copy(v_s[:, cs], vp4[:])
                    yield
                    nc.vector.tensor_scalar_add(qt_s[:, cs], pq[:], bq_ap)
                else:
                    # no Q work to hide behind: drain pv in halves so the
                    # first transposes start after ~half the drain latency
                    nc.vector.tensor_scalar_add(
                        vt_tmp[:, 0:256], pv[:, 0:256], bv_ap
                    )
                    nc.vector.tensor_scalar_add(
                        vt_tmp[:, 256:512], pv[:, 256:512], bv_ap
                    )
                    vp4 = psum.tile([128, 512], bf16, tag="acc512", bufs=2,
                                    name="vp4")
                    transposes(vp4, 0, 2)
                    transposes(vp4, 2, 4)
                    nc.vector.tensor_copy(v_s[:, cs], vp4[:])
                    yield

            def emit_rb(rb, prefetch_rb=None):
                for _ in emit_rb_gen(rb, prefetch_rb):
                    pass

            def build_pairs(phase, u):
                """Pairs of k-tiles sharing one exp: (kt_a, kt_b, midx_a,
                bkey). Masked halves use masks_s tile midx_a + h. Diagonal
                pairs go LAST so the first AV matmul (start=True) always
                covers the full query range."""
                pairs = []
                diag_pairs = []
                for kblk, kind in SLOTS[phase]:
                    tiles = []
                    for t in range(8):
                        if kind == "diag":
                            drel = t * 128 - u * 512
                            if drel >= 512:
                                continue
                            midx = drel // 128 if drel >= 0 else None
                            tiles.append((kblk * 8 + t, midx))
                        else:
                            tiles.append((kblk * 8 + t, None))
                    bkey = kind if kind in slot_bias else None
                    dst = diag_pairs if kind == "diag" else pairs
                    # tiles with masks come in runs of consecutive midx
                    i = 0
                    while i < len(tiles):
                        (ta, ma), (tb, mb) = tiles[i], tiles[i + 1]
                        assert (ma is None) == (mb is None)
                        dst.append((ta, tb, ma, bkey))
                        i += 2
                return pairs + diag_pairs

            def emit_attn_u(phase, u, filler=None, nfill=1):
                q0 = phase * BLK + u * 512
                pairs = build_pairs(phase, u)
                n = len(pairs)
                ngroups = n // 2
                ot_acc = psum.tile([128, 512], f32, tag="otacc", bufs=1,
                                   name="ot_acc")
                rs_acc = psum.tile([1, 512], f32, tag="rs", bufs=1,
                                   name="rs_acc")
                ests = [None] * n

                def emit_pair(pi):
                    ta, tb, ma, bkey = pairs[pi]
                    stp = psum.tile([128, 1024], f32, tag="stp", bufs=2,
                                    name="stp")
                    # causal masks are pre-loaded into PSUM via an identity
                    # matmul and the score matmul accumulates on top — this
                    # keeps the mask off the DVE (a [128,1024] PSUM
                    # tensor_add costs ~1.2us and serializes score->exp).
                    # The score matmul then only covers the live query range
                    # (the mask-init already filled the dead zone with
                    # MASKVAL, so exp drives it to 0).
                    for h, kt in ((0, ta), (1, tb)):
                        if ma is not None:
                            m = ma + h
                            nc.tensor.matmul(
                                stp[:, h * 512:(h + 1) * 512], identb_s[:],
                                masks_s[:, m * 512:(m + 1) * 512],
                                start=True, stop=False,
                                skip_group_check=True,
                            )
                            off = m * 128
                            nc.tensor.matmul(
                                stp[:, h * 512 + off:(h + 1) * 512],
                                kt_s[:, kt * 128:(kt + 1) * 128],
                                qt_s[:, q0 + off:q0 + 512],
                                start=False, stop=True,
                                skip_group_check=True,
                            )
                        else:
                            nc.tensor.matmul(
                                stp[:, h * 512:(h + 1) * 512],
                                kt_s[:, kt * 128:(kt + 1) * 128],
                                qt_s[:, q0:q0 + 512],
                                start=True, stop=True,
                            )
                    est = epool.tile([128, 1024], bf16, tag="est")
                    nc.scalar.activation(
                        est[:], stp[:],
                        mybir.ActivationFunctionType.Exp,
                        bias=slot_bias[bkey] if bkey else 0.0,
                    )
                    ests[pi] = est

                emit_pair(0)
                if n > 1:
                    emit_pair(1)
                for pi in range(n):
                    ta, tb, ma, bkey = pairs[pi]
                    if pi + 2 < n:
                        emit_pair(pi + 2)
                    if filler is not None:
                        for _ in range(nfill):
                            next(filler, None)
                    est = ests[pi]
                    # masked (diagonal) pairs only contribute on their live
                    # query range; they are ordered last so never carry
                    # start=True (the dead columns were already written by
                    # full pairs)
                    for h, kt in ((0, ta), (1, tb)):
                        off = 0 if ma is None else (ma + h) * 128
                        nc.tensor.matmul(
                            ot_acc[:, off:512],
                            v_s[:, kt * 128:(kt + 1) * 128],
                            est[:, h * 512 + off:(h + 1) * 512],
                            start=(pi == 0 and h == 0),
                            stop=(pi == n - 1 and h == 1),
                            skip_group_check=True,
                        )
                    if pi % 2 == 1:
                        g = pi // 2
                        if pi == n - 1:
                            # last group: 4 direct row-sum matmuls instead
                            # of the DVE tree, so the PE carries no DVE
                            # dependency at the unit boundary and the
                            # out-projection can start immediately
                            for j, (e, h) in enumerate(
                                (e, h)
                                for e in (ests[pi - 1], est)
                                for h in (0, 1)
                            ):
                                nc.tensor.matmul(
                                    rs_acc[:], ones_s[:],
                                    e[:, h * 512:(h + 1) * 512],
                                    start=False, stop=(j == 3),
                                )
                        else:
                            esum = epool.tile([128, 1024], bf16, tag="esum",
                                              name="esum")
                            nc.vector.tensor_add(
                                esum[:], ests[pi - 1][:], est[:]
                            )
                            fold = epool.tile([128, 512], bf16, tag="fold",
                                              name="fold")
                            nc.vector.tensor_add(
                                fold[:], esum[:, 0:512], esum[:, 512:1024]
                            )
                            nc.tensor.matmul(
                                rs_acc[:], ones_s[:], fold[:],
                                start=(g == 0), stop=False,
                            )

                qb = phase * 2 + u
                if phase == 1 and u == 1:
                    # last unit: ACT still has queued exps, a split copy
                    # would finish LATER than a single DVE copy
                    nc.vector.tensor_copy(
                        ot_s[:, qb * 512:(qb + 1) * 512], ot_acc[:]
                    )
                else:
                    # split the u-end ot drain across DVE and ACT so the
                    # next consumer (out-proj matmul) unblocks ~2x sooner
                    nc.vector.tensor_copy(
                        ot_s[:, qb * 512:qb * 512 + 256], ot_acc[:, 0:256]
                    )
                    nc.scalar.activation(
                        ot_s[:, qb * 512 + 256:(qb + 1) * 512],
                        ot_acc[:, 256:512],
                        mybir.ActivationFunctionType.Copy,
                    )
                nc.vector.tensor_copy(
                    rs_s[:, qb * 512:(qb + 1) * 512], rs_acc[:]
                )
                if filler is not None:
                    for _ in filler:  # drain unconsumed filler chunks
                        pass

            def outproj_gen(qb, on_act=False, nstores=2, use_stp=False):
                """Out-projection for one 512-query block into a bf16 slab.
                Stores go on the gpsimd SWDGE queue so they never wait
                behind input-prefetch WAR stalls on the sync queue. Yields
                per dt chunk. on_act alternates copies onto ACT (only for
                regions where ACT is not running exp). use_stp borrows the
                stp PSUM banks for a 4-deep drain pipeline — only safe
                after all attention units are done."""
                slab = outpool.tile([128, ND, 512], bf16, tag="oslab",
                                    name="oslab")
                per = ND // nstores
                pop = None
                for dt in range(ND):
                    if use_stp and dt % 4 < 2:
                        # alternate between stp pair-tiles and acc512 so
                        # the drain pipeline is 6 deep after attention
                        if dt % 4 == 0:
                            pop = psum.tile([128, 1024], f32, tag="stp",
                                            bufs=2, name="po")
                        po = pop[:, (dt % 4) * 512:(dt % 4) * 512 + 512]
                    else:
                        po = psum.tile([128, 512], f32, tag="acc512",
                                       bufs=2, name="po")[:]
                    nc.tensor.matmul(
                        po,
                        wo_s[:, dt * 128:(dt + 1) * 128],
                        ot_s[:, qb * 512:(qb + 1) * 512],
                        start=True, stop=True,
                    )
                    if on_act and dt % 2 == 1:
                        nc.scalar.activation(
                            slab[:, dt, :], po,
                            mybir.ActivationFunctionType.Copy,
                        )
                    else:
                        nc.vector.tensor_copy(slab[:, dt, :], po)
                    if dt % per == per - 1:
                        s = dt + 1 - per
                        nc.sync.dma_start(
                            out_d.ap()[:, qb, s:dt + 1], slab[:, s:dt + 1]
                        )
                    yield

            def emit_outproj(qb, on_act=False, nstores=2, use_stp=False):
                for _ in outproj_gen(qb, on_act, nstores, use_stp):
                    pass

            def chain(*gens):
                for g in gens:
                    for x in g:
                        yield x

            def interleave(*gens):
                live = list(gens)
                while live:
                    for g in list(live):
                        try:
                            next(g)
                        except StopIteration:
                            live.remove(g)

            # ---- interleaved schedule ----
            # phase 0 needs local kblks 0 (rbs 0,1) and 2 (rbs 4,5) plus
            # Qt[0:1024) (rbs 0,1); phase 1 needs everything.
            emit_rb(0, prefetch_rb=4)
            nc.sync.dma_start(wo_s[:], wo_d.ap())
            emit_rb(1, prefetch_rb=5)
            emit_rb(4, prefetch_rb=2)
            emit_rb(5, prefetch_rb=3)
            emit_attn_u(0, 0, filler=emit_rb_gen(2, prefetch_rb=6))
            emit_attn_u(0, 1, filler=emit_rb_gen(3, prefetch_rb=7))
            # out-proj of qb0 rides along rb6/rb7: its matmuls interleave
            # with projection matmuls on PE while its PSUM drains use the
            # exp-free DVE/ACT window; attn(1,*) then only carries one
            # out-proj each at 1 chunk/pair so DVE doesn't saturate
            interleave(chain(emit_rb_gen(6), emit_rb_gen(7)),
                       outproj_gen(0, on_act=True))
            emit_attn_u(1, 0, filler=outproj_gen(1), nfill=1)
            emit_attn_u(1, 1, filler=outproj_gen(2), nfill=1)
            nc.sync.dma_start(rs_d.ap(), rs_s[:])
            emit_outproj(3, on_act=True, nstores=4, use_stp=True)

    nc.compile()
    return nc


def _get_program():
    if "nc" not in _cached:
        _cached["nc"] = _build_program()
    return _cached["nc"]


def _perm_blocks(i):
    # local order [qA, qB, o1, o2]
    return [0, 3, 1, 2] if i == 0 else [1, 2, 0, 3]


def _repack_w(w):
    # (D, HD) -> [128, ND, 128] with per-partition contiguous lines
    return np.ascontiguousarray(
        w.reshape(ND, 128, HD).transpose(1, 0, 2)
    ).astype(ml_dtypes.bfloat16)


def make_in_maps(x, Wq, bq, Wk, bk, Wv, bv, Wo, bo):
    scale = 1.0 / np.sqrt(np.float32(HD))
    wq_r = _repack_w((Wq * scale).astype(np.float32))
    wk_r = _repack_w(Wk.astype(np.float32))
    wv_r = _repack_w(Wv.astype(np.float32))
    bq_s = (bq * scale).astype(np.float32)
    in_maps = []
    for c in range(8):
        i, b = c % 2, c // 2
        perm = _perm_blocks(i)
        xbT = x[b].T  # (D, L) view
        xT = np.concatenate(
            [xbT[:, p * BLK:(p + 1) * BLK] for p in perm], axis=1
        )
        # (D, L) -> [128, NRB, ND, 512]: xr[p, rb, dt, c] = xT[dt*128+p,
        # rb*512+c]
        xr = np.ascontiguousarray(
            xT.reshape(ND, 128, NRB, 512).transpose(1, 2, 0, 3)
        ).astype(ml_dtypes.bfloat16)
        biases = np.zeros((128, 8), np.float32)
        biases[:, 0] = bq_s
        biases[:, 1] = bk.astype(np.float32)
        biases[:, 2] = bv.astype(np.float32)
        biases[:, 3] = NEG if i == 0 else 0.0   # phase A, slot kblk=2
        biases[:, 4] = 0.0 if i == 0 else NEG   # phase B, slot kblk=3
        in_maps.append({
            "xr": xr,
            "wq": wq_r,
            "wk": wk_r,
            "wv": wv_r,
            "wo": Wo.astype(ml_dtypes.bfloat16),
            "biases": biases,
        })
    return in_maps


def assemble_output(results, bo):
    out = np.empty((B, L, D), np.float32)
    for c in range(8):
        i, b = c % 2, c // 2
        perm = _perm_blocks(i)
        arr = np.asarray(results[c]["outT"], dtype=np.float32)
        # [128, 4, ND, 512] -> (D, LQ)
        outT = arr.transpose(2, 0, 1, 3).reshape(D, LQ)
        outT /= np.asarray(results[c]["rowsums"], dtype=np.float32)
        qA, qB = perm[0], perm[1]
        out[b, qA * BLK:(qA + 1) * BLK, :] = outT[:, 0:BLK].T
        out[b, qB * BLK:(qB + 1) * BLK, :] = outT[:, BLK:2 * BLK].T
    out += bo.astype(np.float32)
    return out


def kernel(x, Wq, bq, Wk, bk, Wv, bv, Wo, bo):
    from concourse.bass_utils import run_bass_kernel_spmd

    nc = _get_program()
    in_maps = make_in_maps(
        np.asarray(x), np.asarray(Wq), np.asarray(bq), np.asarray(Wk),
        np.asarray(bk), np.asarray(Wv), np.asarray(bv), np.asarray(Wo),
        np.asarray(bo),
    )
    res = run_bass_kernel_spmd(nc, in_maps, core_ids=list(range(8)))
    return assemble_output(res.results, np.asarray(bo))
